# revision 12
# baseline (speedup 1.0000x reference)
"""Trainium2 8-core kernel for the paired contrastive (NT-Xent-like) loss.

Math (tau=0.5, N=8192, D=256):
    z1 = l2norm(H_1), z2 = l2norm(H_2)
    den1_i = sum_j exp(z1.z1/t) + sum_j exp(z1.z2/t) - e^2
    den2_i = sum_j exp(z2.z2/t) + sum_j exp(z2.z1/t) - e^2
    loss = (1/2N) * sum_i [ ln(den1_i) + ln(den2_i) - 2*(z1_i.z2_i)/t ]

v2 scheme (vs. row-parallel baseline):
  * S11/S22 are symmetric: only the upper block-triangle of each is
    computed.  With 1024x1024 blocks the 8x8 grid needs 36 blocks per
    symmetric matrix instead of 64; adding the 64 S12 blocks the global
    work drops 192 -> 136 blocks (exp + matmul both).
  * SPMD block assignment is circulant: core c computes S11 blocks
    (c, c+d mod 8) for d=0..3, S22 blocks likewise, S12 row c (8 blocks),
    plus one "diff-4" block: S11 (c, c+4) for c<4, else S22 (c, c-4) --
    expressed uniformly through per-core input panels (the program is
    identical on every core; only the DRAM contents differ).
  * Off-diagonal symmetric blocks contribute twice: row-sums (own rows,
    kept local via ACT accum_out) and column-sums (other cores' rows).
    Column-sum vectors are AllGather'ed; each core assembles its share
    with a 0/1 selection vector (per-core input data) via one matmul --
    this sidesteps SPMD's compile-time addressing of "(c+d) mod 8".
  * Matmuls run in fp8e4 DoubleRow mode (K=256 in one instruction at
    2x rate); embeddings are scaled by 16 pre-quantization and the exp
    activation scale folds the 1/256 back out.
"""

import math

import numpy as np
import ml_dtypes

import concourse.bass as bass
import concourse.tile as tile
from concourse import bacc, mybir
from concourse.bass_utils import run_bass_kernel_spmd

F32 = mybir.dt.float32
BF16 = mybir.dt.bfloat16
FP8 = mybir.dt.float8e4
AF = mybir.ActivationFunctionType
ALU = mybir.AluOpType
AX = mybir.AxisListType
DR = mybir.MatmulPerfMode.DoubleRow

TAU = 0.5
E2 = math.exp(1.0 / TAU)
QS = 16.0                      # fp8 pre-quantization scale
ESC = 1.0 / (QS * QS * TAU)    # exp() activation scale: undo QS^2, apply 1/tau

N_FULL, D_FULL, N_CORES = 8192, 256, 8
R = N_FULL // N_CORES          # 1024 rows per block
NRT = R // 128                 # 8 row-tiles per block
NV = 15                        # shipped col-sum vectors per core


def build_nc(N=N_FULL, D=D_FULL, n_cores=N_CORES):
    assert (N, D, n_cores) == (N_FULL, D_FULL, N_CORES)
    CH = 512

    nc = bacc.Bacc("TRN2", target_bir_lowering=False, debug=False,
                   num_devices=n_cores)

    ht2 = nc.dram_tensor("HT2", [D, N], BF16, kind="ExternalInput")
    pr1 = nc.dram_tensor("PR1", [D, 4 * R], BF16, kind="ExternalInput")
    pr2 = nc.dram_tensor("PR2", [D, 4 * R], BF16, kind="ExternalInput")
    pdl = nc.dram_tensor("PDL", [D, R], BF16, kind="ExternalInput")
    pdr = nc.dram_tensor("PDR", [D, R], BF16, kind="ExternalInput")
    sel1_in = nc.dram_tensor("SEL1", [128, 1], BF16, kind="ExternalInput")
    sel2_in = nc.dram_tensor("SEL2", [128, 1], BF16, kind="ExternalInput")
    dm1_in = nc.dram_tensor("DM1", [128, 1], F32, kind="ExternalInput")
    dm2_in = nc.dram_tensor("DM2", [128, 1], F32, kind="ExternalInput")
    out = nc.dram_tensor("out", [1, 1], F32, kind="ExternalOutput")

    with tile.TileContext(nc) as tc, \
         tc.tile_pool(name="persist", bufs=1) as per, \
         tc.tile_pool(name="dram", bufs=1, space="DRAM") as dram:
        # --- persistent tensors ---------------------------------------
        z2q = per.tile([128, 2, N], FP8, tag="z2q", name="z2q")
        p1q = per.tile([128, 2, 4 * R], FP8, tag="p1q", name="p1q")
        p2q = per.tile([128, 2, 4 * R], FP8, tag="p2q", name="p2q")
        dlq = per.tile([128, 2, R], FP8, tag="dlq", name="dlq")
        drq = per.tile([128, 2, R], FP8, tag="drq", name="drq")
        ca12 = per.tile([128, N], BF16, tag="ca12", name="ca12")
        ca11 = per.tile([128, 4 * R], BF16, tag="ca11", name="ca11")
        ca22 = per.tile([128, 4 * R], BF16, tag="ca22", name="ca22")
        ca4 = per.tile([128, R], BF16, tag="ca4", name="ca4")
        acc1 = per.tile([128, NRT, 6], F32, tag="acc1", name="acc1")
        acc2 = per.tile([128, NRT, 2], F32, tag="acc2", name="acc2")
        acc4 = per.tile([128, NRT, 1], F32, tag="acc4", name="acc4")
        sel1 = per.tile([128, 1], BF16, tag="sel1", name="sel1")
        sel2 = per.tile([128, 1], BF16, tag="sel2", name="sel2")
        dm1 = per.tile([128, 1], F32, tag="dm1", name="dm1")
        dm2 = per.tile([128, 1], F32, tag="dm2", name="dm2")
        ii_tot = per.tile([1, 1], F32, tag="ii_tot", name="ii_tot")
        lnacc = per.tile([128, 1], F32, tag="lnacc", name="lnacc")
        ones_k = per.tile([128, 1], BF16, tag="ones_k", name="ones_k")
        ones_b = per.tile([1, 128], BF16, tag="ones_b", name="ones_b")
        ones_f = per.tile([128, 1], F32, tag="ones_f", name="ones_f")
        zb = per.tile([128, 1], F32, tag="zb", name="zb")
        ag_in = dram.tile([NV, R], BF16, tag="ag_in", name="ag_in")
        ag_out = dram.tile([8 * NV, R], BF16, tag="ag_out", name="ag_out")
        ds1 = dram.tile([R], F32, tag="ds1", name="ds1")
        ds2 = dram.tile([R], F32, tag="ds2", name="ds2")

        nc.gpsimd.memset(ones_k[:], 1.0)
        nc.gpsimd.memset(ones_b[:], 1.0)
        nc.gpsimd.memset(ones_f[:], 1.0)
        nc.gpsimd.memset(zb[:], 0.0)

        nc.sync.dma_start(sel1[:], sel1_in.ap()[:, :])
        nc.sync.dma_start(sel2[:], sel2_in.ap()[:, :])
        nc.sync.dma_start(dm1[:], dm1_in.ap()[:, :])
        nc.sync.dma_start(dm2[:], dm2_in.ap()[:, :])

        # ============ prep: load, norms, scale+quantize ================
        # specs: (name, dram src, ncols, fp8 dst, ssk row base, dma engines)
        with tc.tile_pool(name="prep", bufs=1) as prep, \
             tc.tile_pool(name="work", bufs=6) as work, \
             tc.tile_pool(name="pps", bufs=2, space="PSUM") as pps:
            p1b = prep.tile([128, 2, 4 * R], BF16, tag="p1b", name="p1b")
            p2b = prep.tile([128, 2, 4 * R], BF16, tag="p2b", name="p2b")
            z2b = prep.tile([128, 2, N], BF16, tag="z2b", name="z2b")
            dlb = prep.tile([128, 2, R], BF16, tag="dlb", name="dlb")
            drb = prep.tile([128, 2, R], BF16, tag="drb", name="drb")
            zi1 = prep.tile([128, 2, R], BF16, tag="zi1", name="zi1")
            zi2 = prep.tile([128, 2, R], BF16, tag="zi2", name="zi2")
            NSS = 36
            ssk = prep.tile([NSS, CH], F32, tag="ssk", name="ssk")
            rvk = prep.tile([NSS, CH], BF16, tag="rvk", name="rvk")
            rvi = prep.tile([4, CH], BF16, tag="rvi", name="rvi")
            rvs = prep.tile([4, CH], F32, tag="rvs", name="rvs")
            nc.gpsimd.memset(ssk[:], 1.0)

            specs = [
                ("pr1", p1b, pr1, 4 * R, p1q, 16, (nc.sync, nc.scalar)),
                ("z2", z2b, ht2, N, z2q, 0, (nc.sync, nc.scalar)),
                ("pr2", p2b, pr2, 4 * R, p2q, 24, (nc.sync, nc.scalar)),
                ("pdl", dlb, pdl, R, dlq, 32, (nc.gpsimd, nc.gpsimd)),
                ("pdr", drb, pdr, R, drq, 34, (nc.gpsimd, nc.gpsimd)),
            ]
            for _, dst, src, ncols, _, _, engs in specs:
                for k in range(2):
                    engs[k].dma_start(dst[:, k, :], src.ap()[bass.ts(k, 128), :])

            # norms: sumsq per column -> 1/sqrt -> *QS (bf16 rows)
            for _, dst, _, ncols, _, srow, _ in specs:
                for c in range(ncols // CH):
                    cs = slice(c * CH, (c + 1) * CH)
                    sq = work.tile([128, 2, CH], BF16, tag="sq", name="sq")
                    nc.vector.tensor_mul(sq[:], dst[:, :, cs], dst[:, :, cs])
                    pn = pps.tile([1, CH], F32, tag="pn", name="pn")
                    for k in range(2):
                        nc.tensor.matmul(pn[:], ones_k[:], sq[:, k, :],
                                         start=(k == 0), stop=(k == 1))
                    stg = work.tile([1, CH], F32, tag="stg", name="stg")
                    nc.vector.tensor_copy(stg[:], pn[:])
                    r = srow + c
                    nc.gpsimd.dma_start(ssk[r:r + 1, :], stg[:])
            nc.vector.reciprocal(ssk[:], ssk[:])
            nc.scalar.activation(ssk[:], ssk[:], AF.Sqrt, bias=zb[:NSS, :])
            nc.vector.tensor_scalar_mul(rvk[:], ssk[:], QS)
            nc.gpsimd.dma_start(rvs[0:2, :], ssk[16:18, :])
            nc.gpsimd.dma_start(rvs[2:4, :], ssk[24:26, :])
            nc.vector.tensor_copy(rvi[:], rvs[:])

            # scale+quantize: z8 = z * rv * QS (fp8), via bcast matmul
            for si, (_, dst, _, ncols, q, srow, _) in enumerate(specs):
                eng = nc.vector
                for c in range(ncols // CH):
                    cs = slice(c * CH, (c + 1) * CH)
                    rst = work.tile([1, 2, CH], BF16, tag="rst", name="rst")
                    r = srow + c
                    nc.gpsimd.dma_start(rst[:, 0, :], rvk[r:r + 1, :])
                    nc.gpsimd.dma_start(rst[:, 1, :], rvk[r:r + 1, :])
                    pb = pps.tile([128, 2, CH], F32, tag="pb", name="pb")
                    for k in range(2):
                        nc.tensor.matmul(pb[:, k, :], ones_b[:], rst[0:1, k, :],
                                         start=True, stop=True)
                    eng.tensor_mul(q[:, :, cs], dst[:, :, cs], pb[:])

            # unit-scale bf16 copies of z1[c], z2[c] rows for the diagonal
            for (srcb, dsti, rb) in ((p1b, zi1, 0), (p2b, zi2, 2)):
                for c in range(R // CH):
                    cs = slice(c * CH, (c + 1) * CH)
                    rst = work.tile([1, 2, CH], BF16, tag="rst", name="rst")
                    nc.gpsimd.dma_start(rst[:, 0, :], rvi[rb + c:rb + c + 1, :])
                    nc.gpsimd.dma_start(rst[:, 1, :], rvi[rb + c:rb + c + 1, :])
                    pb = pps.tile([128, 2, CH], F32, tag="pb", name="pb")
                    for k in range(2):
                        nc.tensor.matmul(pb[:, k, :], ones_b[:], rst[0:1, k, :],
                                         start=True, stop=True)
                    nc.vector.tensor_mul(dsti[:, :, cs], srcb[:, :, cs], pb[:])

            # ii_tot = sum_i z1_i . z2_i over own rows
            prd = prep.tile([128, 2, R], BF16, tag="prd", name="prd")
            nc.vector.tensor_mul(prd[:], zi1[:], zi2[:])
            for c in range(R // CH):
                cs = slice(c * CH, (c + 1) * CH)
                pii = pps.tile([1, CH], F32, tag="pn", name="pn")
                for k in range(2):
                    nc.tensor.matmul(pii[:], ones_k[:], prd[:, k, cs],
                                     start=(k == 0), stop=(k == 1))
                red = work.tile([1, 1], F32, tag="red", name="red")
                nc.vector.tensor_reduce(red[:], pii[:], AX.X, ALU.add)
                if c == 0:
                    nc.vector.tensor_copy(ii_tot[:], red[:])
                else:
                    nc.vector.tensor_add(ii_tot[:], ii_tot[:], red[:])

        # ============ exp streams ======================================
        # stream entries: (LHS fp8, RHS fp8, ncols, colacc, colacc eng,
        #                  acc tile, acc col base, group width)
        streams = [
            ("s11", p1q, p1q, 4 * R, ca11, nc.gpsimd, acc1, 4, 2048),
            ("s12", p1q, z2q, N, ca12, nc.vector, acc1, 0, 2048),
            ("s22", p2q, p2q, 4 * R, ca22, nc.gpsimd, acc2, 0, 2048),
            ("d4", dlq, drq, R, ca4, nc.gpsimd, acc4, 0, 1024),
        ]
        with (
            tc.tile_pool(name="spool", bufs=2, space="PSUM") as spool,
            tc.tile_pool(name="escp", bufs=4) as escp,
            tc.tile_pool(name="agw", bufs=4) as agw,
        ):
            def ship_vec(v, src, coff, cpeng):
                """partition-reduce colacc [128,1024] slice -> ag_in row v."""
                pc = spool.tile([128, 2048], F32, tag="sg", name="sg")
                for h in range(2):
                    nc.tensor.matmul(pc[0:1, h * CH:(h + 1) * CH], ones_k[:],
                                     src[:, coff + h * CH:coff + (h + 1) * CH],
                                     start=True, stop=True)
                agv = agw.tile([1, R], BF16, tag="agv", name="agv")
                cpeng.tensor_copy(agv[:], pc[0:1, 0:R])
                nc.scalar.dma_start(ag_in[v:v + 1, :], agv[:])

            for st, LHS, RHS, ncols, ca, caeng, acc, acb, G in streams:
                ng = ncols // G
                for rt in range(NRT):
                    lhs = LHS[:, :, bass.ts(rt, 128)]
                    for g in range(ng):
                        sg = spool.tile([128, G], F32, tag="sg", name="sg")
                        for h in range(G // CH):
                            col = slice(g * G + h * CH, g * G + (h + 1) * CH)
                            nc.tensor.matmul(sg[:, h * CH:(h + 1) * CH],
                                             lhs, RHS[:, :, col],
                                             start=True, stop=True,
                                             perf_mode=DR)
                        esc = escp.tile([128, G], BF16, tag="esc", name="esc")
                        nc.scalar.activation(
                            esc[:], sg[:], AF.Exp, bias=zb[:], scale=ESC,
                            accum_out=acc[:, rt, acb + g:acb + g + 1])
                        gs = slice(g * G, (g + 1) * G)
                        if rt == 0:
                            caeng.tensor_copy(ca[:, gs], esc[:])
                        else:
                            caeng.tensor_add(ca[:, gs], ca[:, gs], esc[:])
                # stream done: ship its col-sum vectors
                if st == "s11":
                    for d in (1, 2, 3):
                        ship_vec(7 + d, ca11, d * R, nc.vector)
                elif st == "s12":
                    for b in range(8):
                        ship_vec(b, ca12, b * R, nc.vector)
                elif st == "s22":
                    for d in (1, 2, 3):
                        ship_vec(10 + d, ca22, d * R, nc.vector)
                else:
                    ship_vec(14, ca4, 0, nc.vector)

            # ---- collective: gather every core's shipped vectors -----
            nc.gpsimd.collective_compute(
                "AllGather", ALU.bypass,
                replica_groups=[list(range(n_cores))],
                ins=[ag_in.opt()], outs=[ag_out.opt()])

        # ============ selection + final ================================
        with (
            tc.tile_pool(name="fin", bufs=1) as fin,
            tc.tile_pool(name="fps", bufs=2, space="PSUM") as fps,
        ):
            M = fin.tile([128, R], BF16, tag="M", name="M")
            nc.gpsimd.memset(M[96:128, :], 0.0)
            nc.sync.dma_start(M[0:8 * NV, :], ag_out[:, :])
            for i, (seli, dsx) in enumerate(((sel1, ds1), (sel2, ds2))):
                pd = fps.tile([1, R], F32, tag="pd", name="pd")
                for h in range(2):
                    nc.tensor.matmul(pd[:, h * CH:(h + 1) * CH], seli[:],
                                     M[:, h * CH:(h + 1) * CH],
                                     start=True, stop=True)
                sc = fin.tile([1, R], F32, tag=f"sc{i}", name="sc")
                nc.vector.tensor_copy(sc[:], pd[:])
                nc.sync.dma_start(dsx[:], sc[:])
            dn1 = fin.tile([128, NRT], F32, tag="dn1", name="dn1")
            dn2 = fin.tile([128, NRT], F32, tag="dn2", name="dn2")
            nc.sync.dma_start(dn1[:], ds1.rearrange("(t p) -> p t", p=128))
            nc.sync.dma_start(dn2[:], ds2.rearrange("(t p) -> p t", p=128))

            den1 = fin.tile([128, NRT], F32, tag="den1", name="den1")
            den2 = fin.tile([128, NRT], F32, tag="den2", name="den2")
            d4s = fin.tile([128, NRT], F32, tag="d4s", name="d4s")
            for rt in range(NRT):
                nc.vector.tensor_reduce(den1[:, rt:rt + 1], acc1[:, rt, :],
                                        AX.X, ALU.add)
                nc.vector.tensor_reduce(den2[:, rt:rt + 1], acc2[:, rt, :],
                                        AX.X, ALU.add)
                nc.vector.tensor_reduce(d4s[:, rt:rt + 1], acc4[:, rt, :],
                                        AX.X, ALU.add)
            d4m = fin.tile([128, NRT], F32, tag="d4m", name="d4m")
            nc.vector.tensor_scalar_mul(d4m[:], d4s[:], dm1[:])
            nc.vector.tensor_add(den1[:], den1[:], d4m[:])
            nc.vector.tensor_scalar_mul(d4m[:], d4s[:], dm2[:])
            nc.vector.tensor_add(den2[:], den2[:], d4m[:])
            nc.vector.tensor_add(den1[:], den1[:], dn1[:])
            nc.vector.tensor_add(den2[:], den2[:], dn2[:])
            nc.vector.tensor_scalar_add(den1[:], den1[:], -E2)
            nc.vector.tensor_scalar_add(den2[:], den2[:], -E2)

            dd = fin.tile([128, NRT], F32, tag="dd", name="dd")
            nc.vector.tensor_mul(dd[:], den1[:], den2[:])
            lnout = fin.tile([128, NRT], F32, tag="lnout", name="lnout")
            nc.scalar.activation(lnout[:], dd[:], AF.Ln, bias=zb[:],
                                 accum_out=lnacc[:])
            iim = fin.tile([1, 1], F32, tag="iim", name="iim")
            nc.vector.tensor_scalar_mul(iim[:], ii_tot[:], -2.0 / TAU)
            nc.vector.tensor_add(lnacc[0:1, :], lnacc[0:1, :], iim[:])
            ptot = fps.tile([1, 1], F32, tag="ptot", name="ptot")
            nc.tensor.matmul(ptot[:], ones_f[:], lnacc[:], start=True,
                             stop=True)
            res = fin.tile([1, 1], F32, tag="res", name="res")
            nc.vector.tensor_copy(res[:], ptot[:])
            nc.sync.dma_start(out.ap()[:, :], res[:])

    nc.compile()
    return nc


_CACHE = {}


def _compiled(N=N_FULL, D=D_FULL, n_cores=N_CORES):
    key = (N, D, n_cores)
    if key not in _CACHE:
        _CACHE[key] = build_nc(N, D, n_cores)
    return _CACHE[key]


def make_in_maps(H_1, H_2, n_cores=N_CORES):
    H1 = np.asarray(H_1, dtype=np.float32)
    H2 = np.asarray(H_2, dtype=np.float32)
    N = H1.shape[0]
    HT1 = np.ascontiguousarray(H1.astype(ml_dtypes.bfloat16).T)
    HT2 = np.ascontiguousarray(H2.astype(ml_dtypes.bfloat16).T)

    def blk(HT, b):
        return HT[:, (b % 8) * R:(b % 8) * R + R]

    maps = []
    for c in range(n_cores):
        x1 = c < 4  # diff-4 block comes from H1 on cores 0-3, else H2
        HX = HT1 if x1 else HT2
        sel1 = np.zeros(128, np.float32)
        sel2 = np.zeros(128, np.float32)
        for c2 in range(8):
            base = c2 * NV
            for v in range(8):
                if v == c:
                    sel2[base + v] = 1.0
            for di, d in enumerate((1, 2, 3)):
                if (c2 + d) % 8 == c:
                    sel1[base + 8 + di] = 1.0
                    sel2[base + 11 + di] = 1.0
            if (c2 + 4) % 8 == c:
                (sel1 if c2 < 4 else sel2)[base + 14] = 1.0
        maps.append({
            "HT2": HT2,
            "PR1": np.ascontiguousarray(
                np.concatenate([blk(HT1, c + j) for j in range(4)], axis=1)),
            "PR2": np.ascontiguousarray(
                np.concatenate([blk(HT2, c + j) for j in range(4)], axis=1)),
            "PDL": np.ascontiguousarray(blk(HX, c)),
            "PDR": np.ascontiguousarray(blk(HX, c + 4)),
            "SEL1": sel1.astype(ml_dtypes.bfloat16).reshape(128, 1),
            "SEL2": sel2.astype(ml_dtypes.bfloat16).reshape(128, 1),
            "DM1": np.full((128, 1), 1.0 if x1 else 0.0, np.float32),
            "DM2": np.full((128, 1), 0.0 if x1 else 1.0, np.float32),
        })
    return maps


def kernel(H_1, H_2):
    N, D = H_1.shape
    nc = _compiled(N, D, N_CORES)
    in_maps = make_in_maps(H_1, H_2, N_CORES)
    res = run_bass_kernel_spmd(nc, in_maps, core_ids=list(range(N_CORES)))
    total = sum(float(r["out"][0, 0]) for r in res.results)
    return np.float32(total / (2.0 * N))


# revision 19
# speedup vs baseline: 1.0464x; 1.0464x over previous
"""Trainium2 8-core kernel for the paired contrastive (NT-Xent-like) loss.

Math (tau=0.5, N=8192, D=256):
    z1 = l2norm(H_1), z2 = l2norm(H_2)
    den1_i = sum_j exp(z1.z1/t) + sum_j exp(z1.z2/t) - e^2
    den2_i = sum_j exp(z2.z2/t) + sum_j exp(z2.z1/t) - e^2
    loss = (1/2N) * sum_i [ ln(den1_i) + ln(den2_i) - 2*(z1_i.z2_i)/t ]

Scheme:
  * S11/S22 are symmetric: only the upper block-triangle is computed
    (36 of 64 blocks each; with S12's 64 the global work is 136 blocks
    instead of 192 -- exp and matmul both).
  * SPMD circulant assignment: core c computes S11 blocks (c, c+d mod 8)
    d=0..3, S22 likewise, S12 row c, plus one diff-4 block: S11 (c, c+4)
    for c<4 else S22 (c, c-4) -- fed uniformly via per-core input panels.
  * Off-diagonal symmetric blocks contribute row-sums (own rows, local
    via ACT accum_out) and column-sums (other cores' rows).  Column-sum
    vectors are AllGather'ed (split in two so most hides under compute);
    each core assembles its denominators with 0/1 selection vectors
    (per-core input data) via one small matmul, sidestepping SPMD's
    compile-time addressing of "(c+d) mod 8".
  * Matmuls are fp8e4 DoubleRow (K=256 per instruction at 2x rate);
    embeddings are scaled by 16 pre-quantization, the exp activation
    scale folds 1/256 back out.
  * Normalization is pipelined in three phases per tensor -- per-chunk
    sumsq (Pool square + PE ones-matmul), one batched rsqrt across all
    chunks (partition-parallel), per-chunk broadcast+quantize -- and
    software-interleaved with the exp streams.
"""

import math

import numpy as np
import ml_dtypes

import concourse.bass as bass
import concourse.tile as tile
from concourse import bacc, mybir
from concourse.bass_utils import run_bass_kernel_spmd

F32 = mybir.dt.float32
BF16 = mybir.dt.bfloat16
FP8 = mybir.dt.float8e4
AF = mybir.ActivationFunctionType
ALU = mybir.AluOpType
AX = mybir.AxisListType
DR = mybir.MatmulPerfMode.DoubleRow

TAU = 0.5
E2 = math.exp(1.0 / TAU)
QS = 16.0                      # fp8 pre-quantization scale
ESC = 1.0 / (QS * QS * TAU)    # exp() activation scale: undo QS^2, apply 1/tau

N_FULL, D_FULL, N_CORES = 8192, 256, 8
R = N_FULL // N_CORES          # 1024 rows per block
NRT = R // 128                 # 8 row-tiles per block
CH = 512
NV1, NV2 = 11, 4               # AllGather #1 / #2 vector counts per core


def build_nc(N=N_FULL, D=D_FULL, n_cores=N_CORES):
    assert (N, D, n_cores) == (N_FULL, D_FULL, N_CORES)

    nc = bacc.Bacc("TRN2", target_bir_lowering=False, debug=False,
                   num_devices=n_cores)

    ht2 = nc.dram_tensor("HT2", [D, N], BF16, kind="ExternalInput")
    pr1 = nc.dram_tensor("PR1", [D, 4 * R], BF16, kind="ExternalInput")
    pr2 = nc.dram_tensor("PR2", [D, 4 * R], BF16, kind="ExternalInput")
    pdl = nc.dram_tensor("PDL", [D, R], BF16, kind="ExternalInput")
    pdr = nc.dram_tensor("PDR", [D, R], BF16, kind="ExternalInput")
    sel1_in = nc.dram_tensor("SEL1", [128, 1], BF16, kind="ExternalInput")
    sel2_in = nc.dram_tensor("SEL2", [128, 1], BF16, kind="ExternalInput")
    dm1_in = nc.dram_tensor("DM1", [128, 1], F32, kind="ExternalInput")
    dm2_in = nc.dram_tensor("DM2", [128, 1], F32, kind="ExternalInput")
    out = nc.dram_tensor("out", [1, 1], F32, kind="ExternalOutput")

    with tile.TileContext(nc) as tc, \
         tc.tile_pool(name="persist", bufs=1) as per, \
         tc.tile_pool(name="escp", bufs=4) as escp, \
         tc.tile_pool(name="agw", bufs=4) as agw, \
         tc.tile_pool(name="dram", bufs=1, space="DRAM") as dram:
        # --- persistent tensors ---------------------------------------
        z2q = per.tile([128, 2, N], FP8, tag="z2q", name="z2q")
        p1q = per.tile([128, 2, 4 * R], FP8, tag="p1q", name="p1q")
        p2q = per.tile([128, 2, 4 * R], FP8, tag="p2q", name="p2q")
        dlq = per.tile([128, 2, R], FP8, tag="dlq", name="dlq")
        drq = per.tile([128, 2, R], FP8, tag="drq", name="drq")
        ca12 = per.tile([128, N], BF16, tag="ca12", name="ca12")
        ca11 = per.tile([128, 4 * R], BF16, tag="ca11", name="ca11")
        ca22 = per.tile([128, 4 * R], BF16, tag="ca22", name="ca22")
        ca4 = per.tile([128, R], BF16, tag="ca4", name="ca4")
        acc1 = per.tile([128, NRT, 6], F32, tag="acc1", name="acc1")
        acc2 = per.tile([128, NRT, 2], F32, tag="acc2", name="acc2")
        acc4 = per.tile([128, NRT, 1], F32, tag="acc4", name="acc4")
        sel1 = per.tile([128, 1], BF16, tag="sel1", name="sel1")
        sel2 = per.tile([128, 1], BF16, tag="sel2", name="sel2")
        dm1 = per.tile([128, 1], F32, tag="dm1", name="dm1")
        dm2 = per.tile([128, 1], F32, tag="dm2", name="dm2")
        ii_tot = per.tile([1, 1], F32, tag="ii_tot", name="ii_tot")
        lnacc = per.tile([128, 1], F32, tag="lnacc", name="lnacc")
        ones_k = per.tile([128, 1], BF16, tag="ones_k", name="ones_k")
        ones_b = per.tile([1, 128], BF16, tag="ones_b", name="ones_b")
        ones_f = per.tile([128, 1], F32, tag="ones_f", name="ones_f")
        zb = per.tile([128, 1], F32, tag="zb", name="zb")
        ag1_in = dram.tile([NV1, R], BF16, tag="ag1_in", name="ag1_in")
        ag1_out = dram.tile([8 * NV1, R], BF16, tag="ag1_out", name="ag1_out")
        ag2_in = dram.tile([NV2, R], BF16, tag="ag2_in", name="ag2_in")
        ag2_out = dram.tile([8 * NV2, R], BF16, tag="ag2_out", name="ag2_out")
        ds1 = dram.tile([R], F32, tag="ds1", name="ds1")
        ds2 = dram.tile([R], F32, tag="ds2", name="ds2")

        nc.gpsimd.memset(ones_k[:], 1.0)
        nc.gpsimd.memset(ones_b[:], 1.0)
        nc.gpsimd.memset(ones_f[:], 1.0)
        nc.gpsimd.memset(zb[:], 0.0)

        nc.sync.dma_start(sel1[:], sel1_in.ap()[:, :])
        nc.sync.dma_start(sel2[:], sel2_in.ap()[:, :])
        nc.sync.dma_start(dm1[:], dm1_in.ap()[:, :])
        nc.sync.dma_start(dm2[:], dm2_in.ap()[:, :])

        spool_cm = tc.tile_pool(name="spool", bufs=2, space="PSUM")
        spool = spool_cm.__enter__()

        def ship_vec(agt, v, src, coff):
            """partition-reduce colacc [128,1024] slice -> ag row v."""
            pc = spool.tile([128, 2048], F32, tag="sg", name="sg")
            for h in range(2):
                nc.tensor.matmul(pc[0:1, h * CH:(h + 1) * CH], ones_k[:],
                                 src[:, coff + h * CH:coff + (h + 1) * CH],
                                 start=True, stop=True)
            agv = agw.tile([1, R], BF16, tag="agv", name="agv")
            nc.vector.tensor_copy(agv[:], pc[0:1, 0:R])
            nc.scalar.dma_start(agt[v:v + 1, :], agv[:])

        def stream_group(LHS, RHS, rt, g, G, ca, acc, acol, narrow=0):
            """one (row-tile, col-group): matmul+exp+rowsum-accum+colacc.
            narrow: skip colacc on the first `narrow` cols of the group."""
            lhs = LHS[:, :, bass.ts(rt, 128)]
            sg = spool.tile([128, G], F32, tag="sg", name="sg")
            for h in range(G // CH):
                col = slice(g * G + h * CH, g * G + (h + 1) * CH)
                nc.tensor.matmul(sg[:, h * CH:(h + 1) * CH], lhs,
                                 RHS[:, :, col], start=True, stop=True,
                                 perf_mode=DR)
            esc = escp.tile([128, G], BF16, tag="esc", name="esc")
            nc.scalar.activation(esc[:], sg[:], AF.Exp, bias=zb[:], scale=ESC,
                                 accum_out=acc[:, rt, acol:acol + 1])
            if narrow >= G:
                return
            lo = g * G + narrow
            if rt == 0:
                nc.vector.tensor_copy(ca[:, lo:(g + 1) * G], esc[:, narrow:])
            else:
                nc.vector.tensor_add(ca[:, lo:(g + 1) * G],
                                     ca[:, lo:(g + 1) * G], esc[:, narrow:])

        with tc.tile_pool(name="stage", bufs=1) as stg, \
             tc.tile_pool(name="work", bufs=2) as work:
            p1b = stg.tile([128, 2, 4 * R], BF16, tag="p1b", name="p1b")
            p2b = stg.tile([128, 2, 4 * R], BF16, tag="p2b", name="p2b")
            z2b = stg.tile([128, 2, N], BF16, tag="z2b", name="z2b")
            dlb = stg.tile([128, 2, R], BF16, tag="dlb", name="dlb")
            drb = stg.tile([128, 2, R], BF16, tag="drb", name="drb")
            zi1 = stg.tile([128, 2, R], BF16, tag="zi1", name="zi1")
            zi2 = stg.tile([128, 2, R], BF16, tag="zi2", name="zi2")
            rvu = stg.tile([1, 4, CH], F32, tag="rvu", name="rvu")
            ssn = {"p1": stg.tile([8, CH], F32, tag="ssn_p1", name="ssn_p1"),
                   "z2": stg.tile([16, CH], F32, tag="ssn_z2", name="ssn_z2"),
                   "p2": stg.tile([8, CH], F32, tag="ssn_p2", name="ssn_p2"),
                   "dd": stg.tile([4, CH], F32, tag="ssn_dd", name="ssn_dd")}
            rvq = {"p1": stg.tile([8, CH], BF16, tag="rvq_p1", name="rvq_p1"),
                   "z2": stg.tile([16, CH], BF16, tag="rvq_z2",
                                  name="rvq_z2"),
                   "p2": stg.tile([8, CH], BF16, tag="rvq_p2", name="rvq_p2"),
                   "dd": stg.tile([4, CH], BF16, tag="rvq_dd", name="rvq_dd")}

            # --- input loads (pr1 first, z2 split into quarters) -------
            for k in range(2):
                nc.sync.dma_start(p1b[:, k, :], pr1.ap()[bass.ts(k, 128), :])
            for k in range(2):
                for h in range(2):
                    cs = slice(h * N // 2, (h + 1) * N // 2)
                    nc.scalar.dma_start(z2b[:, k, cs],
                                        ht2.ap()[bass.ts(k, 128), cs])
            for k in range(2):
                nc.sync.dma_start(p2b[:, k, :], pr2.ap()[bass.ts(k, 128), :])
            for k in range(2):
                nc.sync.dma_start(dlb[:, k, :], pdl.ap()[bass.ts(k, 128), :])
                nc.sync.dma_start(drb[:, k, :], pdr.ap()[bass.ts(k, 128), :])

            def prep_a(src, key, c, row=None):
                """sumsq of chunk c -> ssn[key] row (Pool square, PE
                ones-matmul into a spool slot, DMA out of PSUM)."""
                cs = slice(c * CH, (c + 1) * CH)
                sq = work.tile([128, 2, CH], BF16, tag="sq", name="sq")
                nc.gpsimd.tensor_mul(sq[:], src[:, :, cs], src[:, :, cs])
                sgp = spool.tile([128, 2048], F32, tag="sg", name="sg")
                pn = sgp[0:1, 0:CH]
                for k in range(2):
                    nc.tensor.matmul(pn, ones_k[:], sq[:, k, :],
                                     start=(k == 0), stop=(k == 1))
                r = c if row is None else row
                st = work.tile([1, CH], F32, tag="st", name="st")
                nc.vector.tensor_copy(st[:], pn)
                nc.sync.dma_start(ssn[key][r:r + 1, :], st[:])

            def prep_b(key, nrows):
                """batched rv = rsqrt(ssn)*QS in bf16 (partition-parallel)."""
                t = ssn[key]
                nc.vector.reciprocal(t[:], t[:])
                nc.scalar.activation(t[:], t[:], AF.Sqrt, bias=zb[:nrows, :])
                nc.vector.tensor_scalar_mul(rvq[key][:], t[:], QS)

            def prep_c(src, q, key, c, row=None):
                """broadcast rv row, quantize chunk c to fp8."""
                cs = slice(c * CH, (c + 1) * CH)
                r = c if row is None else row
                rst = work.tile([1, CH], BF16, tag="rst", name="rst")
                nc.scalar.dma_start(rst[:], rvq[key][r:r + 1, :])
                sgp = spool.tile([128, 2048], F32, tag="sg", name="sg")
                for k in range(2):
                    pb = sgp[:, (1 + k) * CH:(2 + k) * CH]
                    nc.tensor.matmul(pb, ones_b[:], rst[0:1, :],
                                     start=True, stop=True)
                    nc.vector.tensor_mul(q[:, k, cs], src[:, k, cs], pb)

            def unit_chunk(srcb, dsti, rv_row, c):
                """zi[:,:,cs] = bf16 of src * rsqrt (unit scale, diag)."""
                cs = slice(c * CH, (c + 1) * CH)
                rst = work.tile([1, CH], BF16, tag="rst", name="rst")
                nc.vector.tensor_copy(rst[:], rvu[0:1, rv_row, :])
                sgp = spool.tile([128, 2048], F32, tag="sg", name="sg")
                for k in range(2):
                    pb = sgp[:, (1 + k) * CH:(2 + k) * CH]
                    nc.tensor.matmul(pb, ones_b[:], rst[0:1, :],
                                     start=True, stop=True)
                    nc.vector.tensor_mul(dsti[:, k, cs], srcb[:, k, cs], pb)

            def ii_chunk(c):
                """ii_tot += sum over chunk of z1_i.z2_i (own rows)."""
                cs = slice(c * CH, (c + 1) * CH)
                prd = work.tile([128, 2, CH], BF16, tag="sq", name="sq")
                nc.vector.tensor_mul(prd[:], zi1[:, :, cs], zi2[:, :, cs])
                sgp = spool.tile([128, 2048], F32, tag="sg", name="sg")
                pii = sgp[0:1, 0:CH]
                for k in range(2):
                    nc.tensor.matmul(pii, ones_k[:], prd[:, k, :],
                                     start=(k == 0), stop=(k == 1))
                red = work.tile([1, 1], F32, tag="red", name="red")
                nc.vector.tensor_reduce(red[:], pii, AX.X, ALU.add)
                if c == 0:
                    nc.vector.tensor_copy(ii_tot[:], red[:])
                else:
                    nc.vector.tensor_add(ii_tot[:], ii_tot[:], red[:])

            # ---- pr1 head: full normalize of the own-row z1 panel -----
            for c in range(8):
                prep_a(p1b, "p1", c)
            prep_b("p1", 8)
            nc.sync.dma_start(rvu[0:1, 0:2, :], ssn["p1"][0:2, :])
            for c in range(8):
                prep_c(p1b, p1q, "p1", c)

            # ---- s11 stream interleaved with z2 normalize -------------
            s11_groups = [(rt, g) for rt in range(NRT) for g in range(2)]
            for i, (rt, g) in enumerate(s11_groups):
                if i < 8:
                    prep_a(z2b, "z2", 2 * i)
                    prep_a(z2b, "z2", 2 * i + 1)
                elif i == 8:
                    prep_b("z2", 16)
                if i >= 8:
                    prep_c(z2b, z2q, "z2", 2 * (i - 8))
                    prep_c(z2b, z2q, "z2", 2 * (i - 8) + 1)
                stream_group(p1q, p1q, rt, g, 2048, ca11, acc1, 4 + g,
                             narrow=1024 if g == 0 else 0)
            for d in (1, 2, 3):
                ship_vec(ag1_in, 7 + d, ca11, d * R)

            # ---- s12 stream interleaved with pr2/pdl/pdr/diag prep ----
            s12_groups = [(rt, g) for rt in range(NRT) for g in range(4)]
            for i, (rt, g) in enumerate(s12_groups):
                if i < 4:
                    prep_a(p2b, "p2", 2 * i)
                    prep_a(p2b, "p2", 2 * i + 1)
                elif i == 4:
                    prep_b("p2", 8)
                    nc.sync.dma_start(rvu[0:1, 2:4, :], ssn["p2"][0:2, :])
                if 4 <= i < 8:
                    prep_c(p2b, p2q, "p2", 2 * (i - 4))
                    prep_c(p2b, p2q, "p2", 2 * (i - 4) + 1)
                elif i == 8:
                    prep_a(dlb, "dd", 0, row=0)
                    prep_a(dlb, "dd", 1, row=1)
                elif i == 9:
                    prep_a(drb, "dd", 0, row=2)
                    prep_a(drb, "dd", 1, row=3)
                elif i == 10:
                    prep_b("dd", 4)
                elif i == 11:
                    prep_c(dlb, dlq, "dd", 0, row=0)
                    prep_c(dlb, dlq, "dd", 1, row=1)
                elif i == 12:
                    prep_c(drb, drq, "dd", 0, row=2)
                    prep_c(drb, drq, "dd", 1, row=3)
                elif i == 13:
                    unit_chunk(p1b, zi1, 0, 0)
                elif i == 14:
                    unit_chunk(p1b, zi1, 1, 1)
                elif i == 15:
                    unit_chunk(p2b, zi2, 2, 0)
                elif i == 16:
                    unit_chunk(p2b, zi2, 3, 1)
                elif i == 17:
                    ii_chunk(0)
                elif i == 18:
                    ii_chunk(1)
                stream_group(p1q, z2q, rt, g, 2048, ca12, acc1, g)
            for b in range(8):
                ship_vec(ag1_in, b, ca12, b * R)
            nc.gpsimd.collective_compute(
                "AllGather", ALU.bypass,
                replica_groups=[list(range(n_cores))],
                ins=[ag1_in.opt()], outs=[ag1_out.opt()])

        # ---- s22 stream (16 groups) -----------------------------------
        for rt in range(NRT):
            for g in range(2):
                stream_group(p2q, p2q, rt, g, 2048, ca22, acc2, g,
                             narrow=1024 if g == 0 else 0)
        for d in (1, 2, 3):
            ship_vec(ag2_in, d - 1, ca22, d * R)

        # ---- d4 stream (8 groups of 1024) -----------------------------
        for rt in range(NRT):
            stream_group(dlq, drq, rt, 0, 1024, ca4, acc4, 0)
        ship_vec(ag2_in, 3, ca4, 0)
        nc.gpsimd.collective_compute(
            "AllGather", ALU.bypass, replica_groups=[list(range(n_cores))],
            ins=[ag2_in.opt()], outs=[ag2_out.opt()])
        spool_cm.__exit__(None, None, None)

        # ============ selection + final ================================
        with tc.tile_pool(name="fin", bufs=1) as fin, \
             tc.tile_pool(name="fps", bufs=1, space="PSUM") as fps:
            M = fin.tile([128, R], BF16, tag="M", name="M")
            nc.gpsimd.memset(M[96:128, :], 0.0)
            nc.sync.dma_start(M[0:8 * NV1, :], ag1_out[:, :])
            nc.sync.dma_start(M[8 * NV1:8 * (NV1 + NV2), :], ag2_out[:, :])
            for i, (seli, dsx) in enumerate(((sel1, ds1), (sel2, ds2))):
                pd = fps.tile([1, R], F32, tag="pd", name="pd")
                for h in range(2):
                    nc.tensor.matmul(pd[:, h * CH:(h + 1) * CH], seli[:],
                                     M[:, h * CH:(h + 1) * CH],
                                     start=True, stop=True)
                sc = fin.tile([1, R], F32, tag=f"sc{i}", name="sc")
                nc.vector.tensor_copy(sc[:], pd[:])
                nc.sync.dma_start(dsx[:], sc[:])
            dn1 = fin.tile([128, NRT], F32, tag="dn1", name="dn1")
            dn2 = fin.tile([128, NRT], F32, tag="dn2", name="dn2")
            nc.sync.dma_start(dn1[:], ds1.rearrange("(t p) -> p t", p=128))
            nc.sync.dma_start(dn2[:], ds2.rearrange("(t p) -> p t", p=128))

            den1 = fin.tile([128, NRT], F32, tag="den1", name="den1")
            den2 = fin.tile([128, NRT], F32, tag="den2", name="den2")
            d4s = fin.tile([128, NRT], F32, tag="d4s", name="d4s")
            for rt in range(NRT):
                nc.vector.tensor_reduce(den1[:, rt:rt + 1], acc1[:, rt, :],
                                        AX.X, ALU.add)
                nc.vector.tensor_reduce(den2[:, rt:rt + 1], acc2[:, rt, :],
                                        AX.X, ALU.add)
                nc.vector.tensor_reduce(d4s[:, rt:rt + 1], acc4[:, rt, :],
                                        AX.X, ALU.add)
            d4m = fin.tile([128, NRT], F32, tag="d4m", name="d4m")
            nc.vector.tensor_scalar_mul(d4m[:], d4s[:], dm1[:])
            nc.vector.tensor_add(den1[:], den1[:], d4m[:])
            nc.vector.tensor_scalar_mul(d4m[:], d4s[:], dm2[:])
            nc.vector.tensor_add(den2[:], den2[:], d4m[:])
            nc.vector.tensor_add(den1[:], den1[:], dn1[:])
            nc.vector.tensor_add(den2[:], den2[:], dn2[:])
            nc.vector.tensor_scalar_add(den1[:], den1[:], -E2)
            nc.vector.tensor_scalar_add(den2[:], den2[:], -E2)

            dd = fin.tile([128, NRT], F32, tag="dd", name="dd")
            nc.vector.tensor_mul(dd[:], den1[:], den2[:])
            lnout = fin.tile([128, NRT], F32, tag="lnout", name="lnout")
            nc.scalar.activation(lnout[:], dd[:], AF.Ln, bias=zb[:],
                                 accum_out=lnacc[:])
            iim = fin.tile([1, 1], F32, tag="iim", name="iim")
            nc.vector.tensor_scalar_mul(iim[:], ii_tot[:], -2.0 / TAU)
            nc.vector.tensor_add(lnacc[0:1, :], lnacc[0:1, :], iim[:])
            ptot = fps.tile([1, 1], F32, tag="ptot", name="ptot")
            nc.tensor.matmul(ptot[:], ones_f[:], lnacc[:], start=True,
                             stop=True)
            res = fin.tile([1, 1], F32, tag="res", name="res")
            nc.vector.tensor_copy(res[:], ptot[:])
            nc.sync.dma_start(out.ap()[:, :], res[:])

    nc.compile()
    return nc


_CACHE = {}


def _compiled(N=N_FULL, D=D_FULL, n_cores=N_CORES):
    key = (N, D, n_cores)
    if key not in _CACHE:
        _CACHE[key] = build_nc(N, D, n_cores)
    return _CACHE[key]


def make_in_maps(H_1, H_2, n_cores=N_CORES):
    H1 = np.asarray(H_1, dtype=np.float32)
    H2 = np.asarray(H_2, dtype=np.float32)
    HT1 = np.ascontiguousarray(H1.astype(ml_dtypes.bfloat16).T)
    HT2 = np.ascontiguousarray(H2.astype(ml_dtypes.bfloat16).T)

    def blk(HT, b):
        return HT[:, (b % 8) * R:(b % 8) * R + R]

    maps = []
    for c in range(n_cores):
        x1 = c < 4  # diff-4 block comes from H1 on cores 0-3, else H2
        HX = HT1 if x1 else HT2
        sel1 = np.zeros(128, np.float32)
        sel2 = np.zeros(128, np.float32)
        for c2 in range(8):
            b1 = c2 * NV1          # AG1 rows: v 0-7 = s12 col b, 8-10 = s11 d
            sel2[b1 + c] = 1.0
            for di, d in enumerate((1, 2, 3)):
                if (c2 + d) % 8 == c:
                    sel1[b1 + 8 + di] = 1.0
            b2 = 8 * NV1 + c2 * NV2  # AG2 rows: v 0-2 = s22 d, 3 = diff-4
            for di, d in enumerate((1, 2, 3)):
                if (c2 + d) % 8 == c:
                    sel2[b2 + di] = 1.0
            if (c2 + 4) % 8 == c:
                (sel1 if c2 < 4 else sel2)[b2 + 3] = 1.0
        maps.append({
            "HT2": HT2,
            "PR1": np.ascontiguousarray(
                np.concatenate([blk(HT1, c + j) for j in range(4)], axis=1)),
            "PR2": np.ascontiguousarray(
                np.concatenate([blk(HT2, c + j) for j in range(4)], axis=1)),
            "PDL": np.ascontiguousarray(blk(HX, c)),
            "PDR": np.ascontiguousarray(blk(HX, c + 4)),
            "SEL1": sel1.astype(ml_dtypes.bfloat16).reshape(128, 1),
            "SEL2": sel2.astype(ml_dtypes.bfloat16).reshape(128, 1),
            "DM1": np.full((128, 1), 1.0 if x1 else 0.0, np.float32),
            "DM2": np.full((128, 1), 0.0 if x1 else 1.0, np.float32),
        })
    return maps


def kernel(H_1, H_2):
    N, D = H_1.shape
    nc = _compiled(N, D, N_CORES)
    in_maps = make_in_maps(H_1, H_2, N_CORES)
    res = run_bass_kernel_spmd(nc, in_maps, core_ids=list(range(N_CORES)))
    total = sum(float(r["out"][0, 0]) for r in res.results)
    return np.float32(total / (2.0 * N))


# revision 22
# speedup vs baseline: 1.0667x; 1.0194x over previous
"""Trainium2 8-core kernel for the paired contrastive (NT-Xent-like) loss.

Math (tau=0.5, N=8192, D=256):
    z1 = l2norm(H_1), z2 = l2norm(H_2)
    den1_i = sum_j exp(z1.z1/t) + sum_j exp(z1.z2/t) - e^2
    den2_i = sum_j exp(z2.z2/t) + sum_j exp(z2.z1/t) - e^2
    loss = (1/2N) * sum_i [ ln(den1_i) + ln(den2_i) - 2*(z1_i.z2_i)/t ]

Scheme:
  * S11/S22 are symmetric: only the upper block-triangle is computed
    (36 of 64 blocks each; with S12's 64 the global work is 136 blocks
    instead of 192 -- exp and matmul both).
  * SPMD circulant assignment: core c computes S11 blocks (c, c+d mod 8)
    d=0..3, S22 likewise, S12 row c, plus one diff-4 block: S11 (c, c+4)
    for c<4 else S22 (c, c-4) -- fed uniformly via per-core input panels.
  * Off-diagonal symmetric blocks contribute row-sums (own rows, local
    via ACT accum_out) and column-sums (other cores' rows).  Column-sum
    vectors are AllGather'ed (split in two so most hides under compute);
    each core assembles its denominators with 0/1 selection vectors
    (per-core input data) via one small matmul, sidestepping SPMD's
    compile-time addressing of "(c+d) mod 8".
  * Matmuls are fp8e4 DoubleRow (K=256 per instruction at 2x rate);
    embeddings are scaled by 16 pre-quantization, the exp activation
    scale folds 1/256 back out.
  * Normalization is pipelined in three phases per tensor -- per-chunk
    sumsq (Pool square + PE ones-matmul), one batched rsqrt across all
    chunks (partition-parallel), per-chunk broadcast+quantize -- and
    software-interleaved with the exp streams.
"""

import math

import numpy as np
import ml_dtypes

import concourse.bass as bass
import concourse.tile as tile
from concourse import bacc, mybir
from concourse.bass_utils import run_bass_kernel_spmd

F32 = mybir.dt.float32
BF16 = mybir.dt.bfloat16
FP8 = mybir.dt.float8e4
AF = mybir.ActivationFunctionType
ALU = mybir.AluOpType
AX = mybir.AxisListType
DR = mybir.MatmulPerfMode.DoubleRow

TAU = 0.5
E2 = math.exp(1.0 / TAU)
QS = 16.0                      # fp8 pre-quantization scale
ESC = 1.0 / (QS * QS * TAU)    # exp() activation scale: undo QS^2, apply 1/tau

N_FULL, D_FULL, N_CORES = 8192, 256, 8
R = N_FULL // N_CORES          # 1024 rows per block
NRT = R // 128                 # 8 row-tiles per block
CH = 512
NV = 14                        # AllGather vectors per core


def build_nc(N=N_FULL, D=D_FULL, n_cores=N_CORES):
    assert (N, D, n_cores) == (N_FULL, D_FULL, N_CORES)

    nc = bacc.Bacc("TRN2", target_bir_lowering=False, debug=False,
                   num_devices=n_cores)

    ht2 = nc.dram_tensor("HT2", [D, N], BF16, kind="ExternalInput")
    pr1 = nc.dram_tensor("PR1", [D, 4 * R], BF16, kind="ExternalInput")
    pr2 = nc.dram_tensor("PR2", [D, 4 * R], BF16, kind="ExternalInput")
    pd1 = nc.dram_tensor("PDR1", [D, R], BF16, kind="ExternalInput")
    pd2 = nc.dram_tensor("PDR2", [D, R], BF16, kind="ExternalInput")
    sel1_in = nc.dram_tensor("SEL1", [128, 1], BF16, kind="ExternalInput")
    sel2_in = nc.dram_tensor("SEL2", [128, 1], BF16, kind="ExternalInput")
    out = nc.dram_tensor("out", [1, 1], F32, kind="ExternalOutput")

    with tile.TileContext(nc) as tc, \
         tc.tile_pool(name="persist", bufs=1) as per, \
         tc.tile_pool(name="escp", bufs=4) as escp, \
         tc.tile_pool(name="agw", bufs=4) as agw, \
         tc.tile_pool(name="dram", bufs=1, space="DRAM") as dram:
        # --- persistent tensors ---------------------------------------
        z2q = per.tile([128, 2, N], FP8, tag="z2q", name="z2q")
        p1q = per.tile([128, 2, 4 * R], FP8, tag="p1q", name="p1q")
        p2q = per.tile([128, 2, 4 * R], FP8, tag="p2q", name="p2q")
        d1q = per.tile([128, 2, R], FP8, tag="d1q", name="d1q")
        d2q = per.tile([128, 2, R], FP8, tag="d2q", name="d2q")
        ca12 = per.tile([128, N], BF16, tag="ca12", name="ca12")
        ca11 = per.tile([128, 4 * R], BF16, tag="ca11", name="ca11")
        ca22 = per.tile([128, 4 * R], BF16, tag="ca22", name="ca22")
        acc1 = per.tile([128, NRT, 8], F32, tag="acc1", name="acc1")
        acc2 = per.tile([128, NRT, 4], F32, tag="acc2", name="acc2")
        sel1 = per.tile([128, 1], BF16, tag="sel1", name="sel1")
        sel2 = per.tile([128, 1], BF16, tag="sel2", name="sel2")
        ii_tot = per.tile([1, 1], F32, tag="ii_tot", name="ii_tot")
        lnacc = per.tile([128, 1], F32, tag="lnacc", name="lnacc")
        ones_k = per.tile([128, 1], BF16, tag="ones_k", name="ones_k")
        ones_b = per.tile([1, 128], BF16, tag="ones_b", name="ones_b")
        ones_f = per.tile([128, 1], F32, tag="ones_f", name="ones_f")
        zb = per.tile([128, 1], F32, tag="zb", name="zb")
        ag_in = dram.tile([NV, R], BF16, tag="ag_in", name="ag_in")
        ag_out = dram.tile([8 * NV, R], BF16, tag="ag_out", name="ag_out")
        ds1 = dram.tile([R], F32, tag="ds1", name="ds1")
        ds2 = dram.tile([R], F32, tag="ds2", name="ds2")

        nc.gpsimd.memset(ones_k[:], 1.0)
        nc.gpsimd.memset(ones_b[:], 1.0)
        nc.gpsimd.memset(ones_f[:], 1.0)
        nc.gpsimd.memset(zb[:], 0.0)

        nc.sync.dma_start(sel1[:], sel1_in.ap()[:, :])
        nc.sync.dma_start(sel2[:], sel2_in.ap()[:, :])

        spool_cm = tc.tile_pool(name="spool", bufs=2, space="PSUM")
        spool = spool_cm.__enter__()

        def ship_vec(agt, v, src, coff):
            """partition-reduce colacc [128,1024] slice -> ag row v."""
            pc = spool.tile([128, 2048], F32, tag="sg", name="sg")
            for h in range(2):
                nc.tensor.matmul(pc[0:1, h * CH:(h + 1) * CH], ones_k[:],
                                 src[:, coff + h * CH:coff + (h + 1) * CH],
                                 start=True, stop=True)
            agv = agw.tile([1, R], BF16, tag="agv", name="agv")
            nc.vector.tensor_copy(agv[:], pc[0:1, 0:R])
            nc.scalar.dma_start(agt[v:v + 1, :], agv[:])

        def stream_group(LHS, RHS, rt, c0, G, ca, acc, acol):
            """one (row-tile, col-group): matmul+exp+rowsum-accum; colacc
            into ca (None = rowsum-only block)."""
            lhs = LHS[:, :, bass.ts(rt, 128)]
            sg = spool.tile([128, G], F32, tag="sg", name="sg")
            for h in range(G // CH):
                col = slice(c0 + h * CH, c0 + (h + 1) * CH)
                nc.tensor.matmul(sg[:, h * CH:(h + 1) * CH], lhs,
                                 RHS[:, :, col], start=True, stop=True,
                                 perf_mode=DR)
            esc = escp.tile([128, G], BF16, tag="esc", name="esc")
            nc.scalar.activation(esc[:], sg[:], AF.Exp, bias=zb[:], scale=ESC,
                                 accum_out=acc[:, rt, acol:acol + 1])
            if ca is None:
                return
            if rt == 0:
                nc.vector.tensor_copy(ca[:, c0:c0 + G], esc[:])
            else:
                nc.vector.tensor_add(ca[:, c0:c0 + G],
                                     ca[:, c0:c0 + G], esc[:])

        with tc.tile_pool(name="stage", bufs=1) as stg, \
             tc.tile_pool(name="work", bufs=2) as work:
            p1b = stg.tile([128, 2, 4 * R], BF16, tag="p1b", name="p1b")
            p2b = stg.tile([128, 2, 4 * R], BF16, tag="p2b", name="p2b")
            z2b = stg.tile([128, 2, N], BF16, tag="z2b", name="z2b")
            db1 = stg.tile([128, 2, R], BF16, tag="db1", name="db1")
            db2 = stg.tile([128, 2, R], BF16, tag="db2", name="db2")
            zi1 = stg.tile([128, 2, R], BF16, tag="zi1", name="zi1")
            zi2 = stg.tile([128, 2, R], BF16, tag="zi2", name="zi2")
            rvu = stg.tile([1, 4, CH], F32, tag="rvu", name="rvu")
            ssn = {"p1": stg.tile([8, CH], F32, tag="ssn_p1", name="ssn_p1"),
                   "z2": stg.tile([16, CH], F32, tag="ssn_z2", name="ssn_z2"),
                   "p2": stg.tile([8, CH], F32, tag="ssn_p2", name="ssn_p2"),
                   "dd": stg.tile([4, CH], F32, tag="ssn_dd", name="ssn_dd")}
            rvq = {"p1": stg.tile([8, CH], BF16, tag="rvq_p1", name="rvq_p1"),
                   "z2": stg.tile([16, CH], BF16, tag="rvq_z2",
                                  name="rvq_z2"),
                   "p2": stg.tile([8, CH], BF16, tag="rvq_p2", name="rvq_p2"),
                   "dd": stg.tile([4, CH], BF16, tag="rvq_dd", name="rvq_dd")}

            # --- input loads (pr1 first, z2 split into quarters) -------
            for k in range(2):
                nc.sync.dma_start(p1b[:, k, :], pr1.ap()[bass.ts(k, 128), :])
            for k in range(2):
                for h in range(2):
                    cs = slice(h * N // 2, (h + 1) * N // 2)
                    nc.scalar.dma_start(z2b[:, k, cs],
                                        ht2.ap()[bass.ts(k, 128), cs])
            for k in range(2):
                nc.sync.dma_start(p2b[:, k, :], pr2.ap()[bass.ts(k, 128), :])
            for k in range(2):
                nc.sync.dma_start(db1[:, k, :], pd1.ap()[bass.ts(k, 128), :])
                nc.sync.dma_start(db2[:, k, :], pd2.ap()[bass.ts(k, 128), :])

            def prep_a(src, key, c, row=None):
                """sumsq of chunk c -> ssn[key] row (Pool square, PE
                ones-matmul into a spool slot, DMA out of PSUM)."""
                cs = slice(c * CH, (c + 1) * CH)
                sq = work.tile([128, 2, CH], BF16, tag="sq", name="sq")
                nc.gpsimd.tensor_mul(sq[:], src[:, :, cs], src[:, :, cs])
                sgp = spool.tile([128, 2048], F32, tag="sg", name="sg")
                pn = sgp[0:1, 0:CH]
                for k in range(2):
                    nc.tensor.matmul(pn, ones_k[:], sq[:, k, :],
                                     start=(k == 0), stop=(k == 1))
                r = c if row is None else row
                st = work.tile([1, CH], F32, tag="st", name="st")
                nc.vector.tensor_copy(st[:], pn)
                nc.sync.dma_start(ssn[key][r:r + 1, :], st[:])

            def prep_b(key, nrows):
                """batched rv = rsqrt(ssn)*QS in bf16 (partition-parallel)."""
                t = ssn[key]
                nc.vector.reciprocal(t[:], t[:])
                nc.scalar.activation(t[:], t[:], AF.Sqrt, bias=zb[:nrows, :])
                nc.vector.tensor_scalar_mul(rvq[key][:], t[:], QS)

            def prep_c(src, q, key, c, row=None):
                """broadcast rv row, quantize chunk c to fp8."""
                cs = slice(c * CH, (c + 1) * CH)
                r = c if row is None else row
                rst = work.tile([1, CH], BF16, tag="rst", name="rst")
                nc.scalar.dma_start(rst[:], rvq[key][r:r + 1, :])
                sgp = spool.tile([128, 2048], F32, tag="sg", name="sg")
                pb = sgp[:, CH:2 * CH]
                nc.tensor.matmul(pb, ones_b[:], rst[0:1, :],
                                 start=True, stop=True)
                for k in range(2):
                    nc.vector.tensor_mul(q[:, k, cs], src[:, k, cs], pb)

            def unit_chunk(srcb, dsti, rv_row, c):
                """zi[:,:,cs] = bf16 of src * rsqrt (unit scale, diag)."""
                cs = slice(c * CH, (c + 1) * CH)
                rst = work.tile([1, CH], BF16, tag="rst", name="rst")
                nc.vector.tensor_copy(rst[:], rvu[0:1, rv_row, :])
                sgp = spool.tile([128, 2048], F32, tag="sg", name="sg")
                pb = sgp[:, CH:2 * CH]
                nc.tensor.matmul(pb, ones_b[:], rst[0:1, :],
                                 start=True, stop=True)
                for k in range(2):
                    nc.vector.tensor_mul(dsti[:, k, cs], srcb[:, k, cs], pb)

            def ii_chunk(c):
                """ii_tot += sum over chunk of z1_i.z2_i (own rows)."""
                cs = slice(c * CH, (c + 1) * CH)
                prd = work.tile([128, 2, CH], BF16, tag="sq", name="sq")
                nc.vector.tensor_mul(prd[:], zi1[:, :, cs], zi2[:, :, cs])
                sgp = spool.tile([128, 2048], F32, tag="sg", name="sg")
                pii = sgp[0:1, 0:CH]
                for k in range(2):
                    nc.tensor.matmul(pii, ones_k[:], prd[:, k, :],
                                     start=(k == 0), stop=(k == 1))
                red = work.tile([1, 1], F32, tag="red", name="red")
                nc.vector.tensor_reduce(red[:], pii, AX.X, ALU.add)
                if c == 0:
                    nc.vector.tensor_copy(ii_tot[:], red[:])
                else:
                    nc.vector.tensor_add(ii_tot[:], ii_tot[:], red[:])

            # ---- pr1 head: full normalize of the own-row z1 panel -----
            for c in range(8):
                prep_a(p1b, "p1", c)
            prep_b("p1", 8)
            nc.sync.dma_start(rvu[0:1, 0:2, :], ssn["p1"][0:2, :])
            for c in range(8):
                prep_c(p1b, p1q, "p1", c)

            # ---- s11 stream (off-diag) interleaved with z2 normalize --
            s11_groups = [(rt, g) for rt in range(NRT) for g in range(2)]
            for i, (rt, g) in enumerate(s11_groups):
                if i < 8:
                    prep_a(z2b, "z2", 2 * i)
                    prep_a(z2b, "z2", 2 * i + 1)
                elif i == 8:
                    prep_b("z2", 16)
                if i >= 8:
                    prep_c(z2b, z2q, "z2", 2 * (i - 8))
                    prep_c(z2b, z2q, "z2", 2 * (i - 8) + 1)
                if g == 0:
                    stream_group(p1q, p1q, rt, 1024, 2048, ca11, acc1, 4)
                else:
                    stream_group(p1q, p1q, rt, 3072, 1024, ca11, acc1, 5)
            for d in (1, 2, 3):
                ship_vec(ag_in, 7 + d, ca11, d * R)

            # ---- s12 stream interleaved with pr2/pdl/pdr/diag prep ----
            s12_groups = [(rt, g) for rt in range(NRT) for g in range(4)]
            for i, (rt, g) in enumerate(s12_groups):
                if i < 4:
                    prep_a(p2b, "p2", 2 * i)
                    prep_a(p2b, "p2", 2 * i + 1)
                elif i == 4:
                    prep_b("p2", 8)
                    nc.sync.dma_start(rvu[0:1, 2:4, :], ssn["p2"][0:2, :])
                if 4 <= i < 8:
                    prep_c(p2b, p2q, "p2", 2 * (i - 4))
                    prep_c(p2b, p2q, "p2", 2 * (i - 4) + 1)
                elif i == 8:
                    prep_a(db1, "dd", 0, row=0)
                    prep_a(db1, "dd", 1, row=1)
                elif i == 9:
                    prep_a(db2, "dd", 0, row=2)
                    prep_a(db2, "dd", 1, row=3)
                elif i == 10:
                    prep_b("dd", 4)
                elif i == 11:
                    prep_c(db1, d1q, "dd", 0, row=0)
                    prep_c(db1, d1q, "dd", 1, row=1)
                elif i == 12:
                    prep_c(db2, d2q, "dd", 0, row=2)
                    prep_c(db2, d2q, "dd", 1, row=3)
                elif i == 13:
                    unit_chunk(p1b, zi1, 0, 0)
                elif i == 14:
                    unit_chunk(p1b, zi1, 1, 1)
                elif i == 15:
                    unit_chunk(p2b, zi2, 2, 0)
                elif i == 16:
                    unit_chunk(p2b, zi2, 3, 1)
                elif i == 17:
                    ii_chunk(0)
                elif i == 18:
                    ii_chunk(1)
                stream_group(p1q, z2q, rt, g * 2048, 2048, ca12, acc1, g)
            for b in range(8):
                ship_vec(ag_in, b, ca12, b * R)

        # ---- s22 stream (off-diag) ------------------------------------
        for rt in range(NRT):
            stream_group(p2q, p2q, rt, 1024, 2048, ca22, acc2, 0)
            stream_group(p2q, p2q, rt, 3072, 1024, ca22, acc2, 1)
        for d in (1, 2, 3):
            ship_vec(ag_in, 10 + d, ca22, d * R)
        # all 14 vectors shipped: post the single AllGather now, then
        # overlap its latency with the colsum-free local blocks.
        nc.gpsimd.collective_compute(
            "AllGather", ALU.bypass, replica_groups=[list(range(n_cores))],
            ins=[ag_in.opt()], outs=[ag_out.opt()])

        # ---- local rowsum-only blocks: s11/s22 diagonals + both diff-4
        for rt in range(NRT):
            stream_group(p1q, p1q, rt, 0, 1024, None, acc1, 6)
            stream_group(p2q, p2q, rt, 0, 1024, None, acc2, 2)
            stream_group(p1q, d1q, rt, 0, 1024, None, acc1, 7)
            stream_group(p2q, d2q, rt, 0, 1024, None, acc2, 3)
        spool_cm.__exit__(None, None, None)

        # ============ selection + final ================================
        with tc.tile_pool(name="fin", bufs=1) as fin, \
             tc.tile_pool(name="fps", bufs=1, space="PSUM") as fps:
            M = fin.tile([128, R], BF16, tag="M", name="M")
            nc.gpsimd.memset(M[96:128, :], 0.0)
            nc.sync.dma_start(M[0:8 * NV, :], ag_out[:, :])
            for i, (seli, dsx) in enumerate(((sel1, ds1), (sel2, ds2))):
                pd = fps.tile([1, R], F32, tag="pd", name="pd")
                for h in range(2):
                    nc.tensor.matmul(pd[:, h * CH:(h + 1) * CH], seli[:],
                                     M[:, h * CH:(h + 1) * CH],
                                     start=True, stop=True)
                sc = fin.tile([1, R], F32, tag=f"sc{i}", name="sc")
                nc.vector.tensor_copy(sc[:], pd[:])
                nc.sync.dma_start(dsx[:], sc[:])
            dn1 = fin.tile([128, NRT], F32, tag="dn1", name="dn1")
            dn2 = fin.tile([128, NRT], F32, tag="dn2", name="dn2")
            nc.sync.dma_start(dn1[:], ds1.rearrange("(t p) -> p t", p=128))
            nc.sync.dma_start(dn2[:], ds2.rearrange("(t p) -> p t", p=128))

            den1 = fin.tile([128, NRT], F32, tag="den1", name="den1")
            den2 = fin.tile([128, NRT], F32, tag="den2", name="den2")
            for rt in range(NRT):
                nc.vector.tensor_reduce(den1[:, rt:rt + 1], acc1[:, rt, :],
                                        AX.X, ALU.add)
                nc.vector.tensor_reduce(den2[:, rt:rt + 1], acc2[:, rt, :],
                                        AX.X, ALU.add)
            nc.vector.tensor_add(den1[:], den1[:], dn1[:])
            nc.vector.tensor_add(den2[:], den2[:], dn2[:])
            nc.vector.tensor_scalar_add(den1[:], den1[:], -E2)
            nc.vector.tensor_scalar_add(den2[:], den2[:], -E2)

            dd = fin.tile([128, NRT], F32, tag="dd", name="dd")
            nc.vector.tensor_mul(dd[:], den1[:], den2[:])
            lnout = fin.tile([128, NRT], F32, tag="lnout", name="lnout")
            nc.scalar.activation(lnout[:], dd[:], AF.Ln, bias=zb[:],
                                 accum_out=lnacc[:])
            iim = fin.tile([1, 1], F32, tag="iim", name="iim")
            nc.vector.tensor_scalar_mul(iim[:], ii_tot[:], -2.0 / TAU)
            nc.vector.tensor_add(lnacc[0:1, :], lnacc[0:1, :], iim[:])
            ptot = fps.tile([1, 1], F32, tag="ptot", name="ptot")
            nc.tensor.matmul(ptot[:], ones_f[:], lnacc[:], start=True,
                             stop=True)
            res = fin.tile([1, 1], F32, tag="res", name="res")
            nc.vector.tensor_copy(res[:], ptot[:])
            nc.sync.dma_start(out.ap()[:, :], res[:])

    nc.compile()
    return nc


_CACHE = {}


def _compiled(N=N_FULL, D=D_FULL, n_cores=N_CORES):
    key = (N, D, n_cores)
    if key not in _CACHE:
        _CACHE[key] = build_nc(N, D, n_cores)
    return _CACHE[key]


def make_in_maps(H_1, H_2, n_cores=N_CORES):
    H1 = np.asarray(H_1, dtype=np.float32)
    H2 = np.asarray(H_2, dtype=np.float32)
    HT1 = np.ascontiguousarray(H1.astype(ml_dtypes.bfloat16).T)
    HT2 = np.ascontiguousarray(H2.astype(ml_dtypes.bfloat16).T)

    def blk(HT, b):
        return HT[:, (b % 8) * R:(b % 8) * R + R]

    maps = []
    for c in range(n_cores):
        sel1 = np.zeros(128, np.float32)
        sel2 = np.zeros(128, np.float32)
        for c2 in range(8):
            b1 = c2 * NV  # rows: v 0-7 = s12 col b, 8-10 = s11 d, 11-13 = s22
            sel2[b1 + c] = 1.0
            for di, d in enumerate((1, 2, 3)):
                if (c2 + d) % 8 == c:
                    sel1[b1 + 8 + di] = 1.0
                    sel2[b1 + 11 + di] = 1.0
        maps.append({
            "HT2": HT2,
            "PR1": np.ascontiguousarray(
                np.concatenate([blk(HT1, c + j) for j in range(4)], axis=1)),
            "PR2": np.ascontiguousarray(
                np.concatenate([blk(HT2, c + j) for j in range(4)], axis=1)),
            "PDR1": np.ascontiguousarray(blk(HT1, c + 4)),
            "PDR2": np.ascontiguousarray(blk(HT2, c + 4)),
            "SEL1": sel1.astype(ml_dtypes.bfloat16).reshape(128, 1),
            "SEL2": sel2.astype(ml_dtypes.bfloat16).reshape(128, 1),
        })
    return maps


def kernel(H_1, H_2):
    N, D = H_1.shape
    nc = _compiled(N, D, N_CORES)
    in_maps = make_in_maps(H_1, H_2, N_CORES)
    res = run_bass_kernel_spmd(nc, in_maps, core_ids=list(range(N_CORES)))
    total = sum(float(r["out"][0, 0]) for r in res.results)
    return np.float32(total / (2.0 * N))


# revision 23
# speedup vs baseline: 1.1545x; 1.0822x over previous
"""Trainium2 8-core kernel for the paired contrastive (NT-Xent-like) loss.

Math (tau=0.5, N=8192, D=256):
    z1 = l2norm(H_1), z2 = l2norm(H_2)
    den1_i = sum_j exp(z1.z1/t) + sum_j exp(z1.z2/t) - e^2
    den2_i = sum_j exp(z2.z2/t) + sum_j exp(z2.z1/t) - e^2
    loss = (1/2N) * sum_i [ ln(den1_i) + ln(den2_i) - 2*(z1_i.z2_i)/t ]

Scheme:
  * S11/S22 are symmetric: only the upper block-triangle is computed
    (36 of 64 blocks each; with S12's 64 the global work is 136 blocks
    instead of 192 -- exp and matmul both).
  * SPMD circulant assignment: core c computes S11 blocks (c, c+d mod 8)
    d=0..3, S22 likewise, S12 row c, plus one diff-4 block: S11 (c, c+4)
    for c<4 else S22 (c, c-4) -- fed uniformly via per-core input panels.
  * Off-diagonal symmetric blocks contribute row-sums (own rows, local
    via ACT accum_out) and column-sums (other cores' rows).  Column-sum
    vectors are AllGather'ed (split in two so most hides under compute);
    each core assembles its denominators with 0/1 selection vectors
    (per-core input data) via one small matmul, sidestepping SPMD's
    compile-time addressing of "(c+d) mod 8".
  * Matmuls are fp8e4 DoubleRow (K=256 per instruction at 2x rate);
    embeddings are scaled by 16 pre-quantization, the exp activation
    scale folds 1/256 back out.
  * Normalization is pipelined in three phases per tensor -- per-chunk
    sumsq (Pool square + PE ones-matmul), one batched rsqrt across all
    chunks (partition-parallel), per-chunk broadcast+quantize -- and
    software-interleaved with the exp streams.
"""

import math

import numpy as np
import ml_dtypes

import concourse.bass as bass
import concourse.tile as tile
from concourse import bacc, mybir
from concourse.bass_utils import run_bass_kernel_spmd

F32 = mybir.dt.float32
BF16 = mybir.dt.bfloat16
FP8 = mybir.dt.float8e4
AF = mybir.ActivationFunctionType
ALU = mybir.AluOpType
AX = mybir.AxisListType
DR = mybir.MatmulPerfMode.DoubleRow

TAU = 0.5
E2 = math.exp(1.0 / TAU)
QS = 16.0                      # fp8 pre-quantization scale
ESC = 1.0 / (QS * QS * TAU)    # exp() activation scale: undo QS^2, apply 1/tau

N_FULL, D_FULL, N_CORES = 8192, 256, 8
R = N_FULL // N_CORES          # 1024 rows per block
NRT = R // 128                 # 8 row-tiles per block
CH = 512
NV = 14                        # AllGather vectors per core


def build_nc(N=N_FULL, D=D_FULL, n_cores=N_CORES):
    assert (N, D, n_cores) == (N_FULL, D_FULL, N_CORES)

    nc = bacc.Bacc("TRN2", target_bir_lowering=False, debug=False,
                   num_devices=n_cores)

    ht2 = nc.dram_tensor("HT2", [D, N], BF16, kind="ExternalInput")
    pr1 = nc.dram_tensor("PR1", [D, 4 * R], BF16, kind="ExternalInput")
    pr2 = nc.dram_tensor("PR2", [D, 4 * R], BF16, kind="ExternalInput")
    pd1 = nc.dram_tensor("PDR1", [D, R], BF16, kind="ExternalInput")
    pd2 = nc.dram_tensor("PDR2", [D, R], BF16, kind="ExternalInput")
    sel1_in = nc.dram_tensor("SEL1", [128, 1], BF16, kind="ExternalInput")
    sel2_in = nc.dram_tensor("SEL2", [128, 1], BF16, kind="ExternalInput")
    out = nc.dram_tensor("out", [1, 1], F32, kind="ExternalOutput")

    with tile.TileContext(nc) as tc, \
         tc.tile_pool(name="persist", bufs=1) as per, \
         tc.tile_pool(name="escp", bufs=4) as escp, \
         tc.tile_pool(name="agw", bufs=4) as agw, \
         tc.tile_pool(name="dram", bufs=1, space="DRAM") as dram:
        # --- persistent tensors ---------------------------------------
        z2q = per.tile([128, 2, N], FP8, tag="z2q", name="z2q")
        p1q = per.tile([128, 2, 4 * R], FP8, tag="p1q", name="p1q")
        p2q = per.tile([128, 2, 4 * R], FP8, tag="p2q", name="p2q")
        d1q = per.tile([128, 2, R], FP8, tag="d1q", name="d1q")
        d2q = per.tile([128, 2, R], FP8, tag="d2q", name="d2q")
        ca12 = per.tile([128, N], BF16, tag="ca12", name="ca12")
        ca11 = per.tile([128, 4 * R], BF16, tag="ca11", name="ca11")
        ca22 = per.tile([128, 4 * R], BF16, tag="ca22", name="ca22")
        acc1 = per.tile([128, NRT, 8], F32, tag="acc1", name="acc1")
        acc2 = per.tile([128, NRT, 4], F32, tag="acc2", name="acc2")
        sel1 = per.tile([128, 1], BF16, tag="sel1", name="sel1")
        sel2 = per.tile([128, 1], BF16, tag="sel2", name="sel2")
        ii_tot = per.tile([1, 1], F32, tag="ii_tot", name="ii_tot")
        lnacc = per.tile([128, 1], F32, tag="lnacc", name="lnacc")
        ones_k = per.tile([128, 1], BF16, tag="ones_k", name="ones_k")
        ones_b = per.tile([1, 128], BF16, tag="ones_b", name="ones_b")
        ones_f = per.tile([128, 1], F32, tag="ones_f", name="ones_f")
        zb = per.tile([128, 1], F32, tag="zb", name="zb")
        ag_in = dram.tile([NV, R], BF16, tag="ag_in", name="ag_in")
        ag_out = dram.tile([8 * NV, R], BF16, tag="ag_out", name="ag_out")
        ds1 = dram.tile([R], F32, tag="ds1", name="ds1")
        ds2 = dram.tile([R], F32, tag="ds2", name="ds2")

        nc.gpsimd.memset(ones_k[:], 1.0)
        nc.gpsimd.memset(ones_b[:], 1.0)
        nc.gpsimd.memset(ones_f[:], 1.0)
        nc.gpsimd.memset(zb[:], 0.0)

        nc.sync.dma_start(sel1[:], sel1_in.ap()[:, :])
        nc.sync.dma_start(sel2[:], sel2_in.ap()[:, :])

        spool_cm = tc.tile_pool(name="spool", bufs=2, space="PSUM")
        spool = spool_cm.__enter__()

        def ship_vec(agt, v, src, coff):
            """partition-reduce colacc [128,1024] slice -> ag row v."""
            pc = spool.tile([128, 2048], F32, tag="sg", name="sg")
            for h in range(2):
                nc.tensor.matmul(pc[0:1, h * CH:(h + 1) * CH], ones_k[:],
                                 src[:, coff + h * CH:coff + (h + 1) * CH],
                                 start=True, stop=True)
            agv = agw.tile([1, R], BF16, tag="agv", name="agv")
            nc.vector.tensor_copy(agv[:], pc[0:1, 0:R])
            nc.scalar.dma_start(agt[v:v + 1, :], agv[:])

        def stream_group(LHS, RHS, rt, c0, G, ca, acc, acol):
            """one (row-tile, col-group): matmul+exp+rowsum-accum; colacc
            into ca (None = rowsum-only block)."""
            lhs = LHS[:, :, bass.ts(rt, 128)]
            sg = spool.tile([128, G], F32, tag="sg", name="sg")
            for h in range(G // CH):
                col = slice(c0 + h * CH, c0 + (h + 1) * CH)
                nc.tensor.matmul(sg[:, h * CH:(h + 1) * CH], lhs,
                                 RHS[:, :, col], start=True, stop=True,
                                 perf_mode=DR)
            esc = escp.tile([128, G], BF16, tag="esc", name="esc")
            nc.scalar.activation(esc[:], sg[:], AF.Exp, bias=zb[:], scale=ESC,
                                 accum_out=acc[:, rt, acol:acol + 1])
            if ca is None:
                return
            if rt == 0:
                nc.vector.tensor_copy(ca[:, c0:c0 + G], esc[:])
            else:
                nc.vector.tensor_add(ca[:, c0:c0 + G],
                                     ca[:, c0:c0 + G], esc[:])

        with tc.tile_pool(name="stage", bufs=1) as stg, \
             tc.tile_pool(name="work", bufs=2) as work:
            p1b = stg.tile([128, 2, 4 * R], BF16, tag="p1b", name="p1b")
            p2b = stg.tile([128, 2, 4 * R], BF16, tag="p2b", name="p2b")
            z2b = stg.tile([128, 2, N], BF16, tag="z2b", name="z2b")
            db1 = stg.tile([128, 2, R], BF16, tag="db1", name="db1")
            db2 = stg.tile([128, 2, R], BF16, tag="db2", name="db2")
            zi1 = stg.tile([128, 2, R], BF16, tag="zi1", name="zi1")
            zi2 = stg.tile([128, 2, R], BF16, tag="zi2", name="zi2")
            rvu = stg.tile([1, 4, CH], F32, tag="rvu", name="rvu")
            ssn = {"p1": stg.tile([8, CH], F32, tag="ssn_p1", name="ssn_p1"),
                   "z2": stg.tile([16, CH], F32, tag="ssn_z2", name="ssn_z2"),
                   "p2": stg.tile([8, CH], F32, tag="ssn_p2", name="ssn_p2"),
                   "dd": stg.tile([4, CH], F32, tag="ssn_dd", name="ssn_dd")}
            rvq = {"p1": stg.tile([8, CH], BF16, tag="rvq_p1", name="rvq_p1"),
                   "z2": stg.tile([16, CH], BF16, tag="rvq_z2",
                                  name="rvq_z2"),
                   "p2": stg.tile([8, CH], BF16, tag="rvq_p2", name="rvq_p2"),
                   "dd": stg.tile([4, CH], BF16, tag="rvq_dd", name="rvq_dd")}

            # --- input loads (pr1 first, z2 split into quarters) -------
            for k in range(2):
                nc.sync.dma_start(p1b[:, k, :], pr1.ap()[bass.ts(k, 128), :])
            for k in range(2):
                for h in range(2):
                    cs = slice(h * N // 2, (h + 1) * N // 2)
                    nc.scalar.dma_start(z2b[:, k, cs],
                                        ht2.ap()[bass.ts(k, 128), cs])
            for k in range(2):
                nc.sync.dma_start(p2b[:, k, :], pr2.ap()[bass.ts(k, 128), :])
            for k in range(2):
                nc.sync.dma_start(db1[:, k, :], pd1.ap()[bass.ts(k, 128), :])
                nc.sync.dma_start(db2[:, k, :], pd2.ap()[bass.ts(k, 128), :])

            def prep_a(src, key, c, row=None, sqeng=None):
                """sumsq of chunk c -> ssn[key] row (square on sqeng, PE
                ones-matmul into a spool slot, copy+DMA out of PSUM)."""
                cs = slice(c * CH, (c + 1) * CH)
                sq = work.tile([128, 2, CH], BF16, tag="sq", name="sq")
                (sqeng or nc.gpsimd).tensor_mul(sq[:], src[:, :, cs],
                                                src[:, :, cs])
                sgp = spool.tile([128, 2048], F32, tag="sg", name="sg")
                pn = sgp[0:1, 0:CH]
                for k in range(2):
                    nc.tensor.matmul(pn, ones_k[:], sq[:, k, :],
                                     start=(k == 0), stop=(k == 1))
                r = c if row is None else row
                st = work.tile([1, CH], F32, tag="st", name="st")
                nc.vector.tensor_copy(st[:], pn)
                nc.sync.dma_start(ssn[key][r:r + 1, :], st[:])

            def prep_b(key, nrows):
                """batched rv = rsqrt(ssn)*QS in bf16 (partition-parallel)."""
                t = ssn[key]
                nc.vector.reciprocal(t[:], t[:])
                nc.scalar.activation(t[:], t[:], AF.Sqrt, bias=zb[:nrows, :])
                nc.vector.tensor_scalar_mul(rvq[key][:], t[:], QS)

            def prep_c(src, q, key, c, row=None):
                """broadcast rv row, quantize chunk c to fp8."""
                cs = slice(c * CH, (c + 1) * CH)
                r = c if row is None else row
                rst = work.tile([1, CH], BF16, tag="rst", name="rst")
                nc.scalar.dma_start(rst[:], rvq[key][r:r + 1, :])
                sgp = spool.tile([128, 2048], F32, tag="sg", name="sg")
                pb = sgp[:, CH:2 * CH]
                nc.tensor.matmul(pb, ones_b[:], rst[0:1, :],
                                 start=True, stop=True)
                for k in range(2):
                    nc.vector.tensor_mul(q[:, k, cs], src[:, k, cs], pb)

            def unit_chunk(srcb, dsti, rv_row, c):
                """zi[:,:,cs] = bf16 of src * rsqrt (unit scale, diag)."""
                cs = slice(c * CH, (c + 1) * CH)
                rst = work.tile([1, CH], BF16, tag="rst", name="rst")
                nc.vector.tensor_copy(rst[:], rvu[0:1, rv_row, :])
                sgp = spool.tile([128, 2048], F32, tag="sg", name="sg")
                pb = sgp[:, CH:2 * CH]
                nc.tensor.matmul(pb, ones_b[:], rst[0:1, :],
                                 start=True, stop=True)
                for k in range(2):
                    nc.vector.tensor_mul(dsti[:, k, cs], srcb[:, k, cs], pb)

            def ii_chunk(c):
                """ii_tot += sum over chunk of z1_i.z2_i (own rows)."""
                cs = slice(c * CH, (c + 1) * CH)
                prd = work.tile([128, 2, CH], BF16, tag="sq", name="sq")
                nc.vector.tensor_mul(prd[:], zi1[:, :, cs], zi2[:, :, cs])
                sgp = spool.tile([128, 2048], F32, tag="sg", name="sg")
                pii = sgp[0:1, 0:CH]
                for k in range(2):
                    nc.tensor.matmul(pii, ones_k[:], prd[:, k, :],
                                     start=(k == 0), stop=(k == 1))
                red = work.tile([1, 1], F32, tag="red", name="red")
                nc.vector.tensor_reduce(red[:], pii, AX.X, ALU.add)
                if c == 0:
                    nc.vector.tensor_copy(ii_tot[:], red[:])
                else:
                    nc.vector.tensor_add(ii_tot[:], ii_tot[:], red[:])

            # ---- pr1 head: full normalize of the own-row z1 panel -----
            for c in range(8):
                prep_a(p1b, "p1", c, sqeng=nc.vector)
            prep_b("p1", 8)
            nc.sync.dma_start(rvu[0:1, 0:2, :], ssn["p1"][0:2, :])
            for c in range(8):
                prep_c(p1b, p1q, "p1", c)

            # ---- s11 stream (off-diag) interleaved with z2 normalize --
            s11_groups = [(rt, g) for rt in range(NRT) for g in range(2)]
            for i, (rt, g) in enumerate(s11_groups):
                if i < 8:
                    prep_a(z2b, "z2", 2 * i, sqeng=nc.vector)
                    prep_a(z2b, "z2", 2 * i + 1, sqeng=nc.gpsimd)
                elif i == 8:
                    prep_b("z2", 16)
                if i >= 8:
                    prep_c(z2b, z2q, "z2", 2 * (i - 8))
                    prep_c(z2b, z2q, "z2", 2 * (i - 8) + 1)
                if g == 0:
                    stream_group(p1q, p1q, rt, 1024, 2048, ca11, acc1, 4)
                else:
                    stream_group(p1q, p1q, rt, 3072, 1024, ca11, acc1, 5)
            for d in (1, 2, 3):
                ship_vec(ag_in, 7 + d, ca11, d * R)

            # ---- s12 stream interleaved with pr2/pdl/pdr/diag prep ----
            s12_groups = [(rt, g) for rt in range(NRT) for g in range(4)]
            for i, (rt, g) in enumerate(s12_groups):
                if i < 4:
                    prep_a(p2b, "p2", 2 * i, sqeng=nc.vector)
                    prep_a(p2b, "p2", 2 * i + 1, sqeng=nc.gpsimd)
                elif i == 4:
                    prep_b("p2", 8)
                    nc.sync.dma_start(rvu[0:1, 2:4, :], ssn["p2"][0:2, :])
                if 4 <= i < 8:
                    prep_c(p2b, p2q, "p2", 2 * (i - 4))
                    prep_c(p2b, p2q, "p2", 2 * (i - 4) + 1)
                elif i == 8:
                    prep_a(db1, "dd", 0, row=0)
                    prep_a(db1, "dd", 1, row=1)
                elif i == 9:
                    prep_a(db2, "dd", 0, row=2)
                    prep_a(db2, "dd", 1, row=3)
                elif i == 10:
                    prep_b("dd", 4)
                elif i == 11:
                    prep_c(db1, d1q, "dd", 0, row=0)
                    prep_c(db1, d1q, "dd", 1, row=1)
                elif i == 12:
                    prep_c(db2, d2q, "dd", 0, row=2)
                    prep_c(db2, d2q, "dd", 1, row=3)
                elif i == 13:
                    unit_chunk(p1b, zi1, 0, 0)
                elif i == 14:
                    unit_chunk(p1b, zi1, 1, 1)
                elif i == 15:
                    unit_chunk(p2b, zi2, 2, 0)
                elif i == 16:
                    unit_chunk(p2b, zi2, 3, 1)
                elif i == 17:
                    ii_chunk(0)
                elif i == 18:
                    ii_chunk(1)
                stream_group(p1q, z2q, rt, g * 2048, 2048, ca12, acc1, g)
            for b in range(8):
                ship_vec(ag_in, b, ca12, b * R)

        # ---- s22 stream (off-diag) ------------------------------------
        for rt in range(NRT):
            stream_group(p2q, p2q, rt, 1024, 2048, ca22, acc2, 0)
            stream_group(p2q, p2q, rt, 3072, 1024, ca22, acc2, 1)
        for d in (1, 2, 3):
            ship_vec(ag_in, 10 + d, ca22, d * R)
        # all 14 vectors shipped: post the single AllGather now, then
        # overlap its latency with the colsum-free local blocks.
        nc.gpsimd.collective_compute(
            "AllGather", ALU.bypass, replica_groups=[list(range(n_cores))],
            ins=[ag_in.opt()], outs=[ag_out.opt()])

        # ---- local rowsum-only blocks: s11/s22 diagonals + both diff-4
        for rt in range(NRT):
            stream_group(p1q, p1q, rt, 0, 1024, None, acc1, 6)
            stream_group(p2q, p2q, rt, 0, 1024, None, acc2, 2)
            stream_group(p1q, d1q, rt, 0, 1024, None, acc1, 7)
            stream_group(p2q, d2q, rt, 0, 1024, None, acc2, 3)
        spool_cm.__exit__(None, None, None)

        # ============ selection + final ================================
        with tc.tile_pool(name="fin", bufs=1) as fin, \
             tc.tile_pool(name="fps", bufs=1, space="PSUM") as fps:
            M = fin.tile([128, R], BF16, tag="M", name="M")
            nc.gpsimd.memset(M[96:128, :], 0.0)
            nc.sync.dma_start(M[0:8 * NV, :], ag_out[:, :])
            for i, (seli, dsx) in enumerate(((sel1, ds1), (sel2, ds2))):
                pd = fps.tile([1, R], F32, tag="pd", name="pd")
                for h in range(2):
                    nc.tensor.matmul(pd[:, h * CH:(h + 1) * CH], seli[:],
                                     M[:, h * CH:(h + 1) * CH],
                                     start=True, stop=True)
                sc = fin.tile([1, R], F32, tag=f"sc{i}", name="sc")
                nc.vector.tensor_copy(sc[:], pd[:])
                nc.sync.dma_start(dsx[:], sc[:])
            dn1 = fin.tile([128, NRT], F32, tag="dn1", name="dn1")
            dn2 = fin.tile([128, NRT], F32, tag="dn2", name="dn2")
            nc.sync.dma_start(dn1[:], ds1.rearrange("(t p) -> p t", p=128))
            nc.sync.dma_start(dn2[:], ds2.rearrange("(t p) -> p t", p=128))

            den1 = fin.tile([128, NRT], F32, tag="den1", name="den1")
            den2 = fin.tile([128, NRT], F32, tag="den2", name="den2")
            for rt in range(NRT):
                nc.vector.tensor_reduce(den1[:, rt:rt + 1], acc1[:, rt, :],
                                        AX.X, ALU.add)
                nc.vector.tensor_reduce(den2[:, rt:rt + 1], acc2[:, rt, :],
                                        AX.X, ALU.add)
            nc.vector.tensor_add(den1[:], den1[:], dn1[:])
            nc.vector.tensor_add(den2[:], den2[:], dn2[:])
            nc.vector.tensor_scalar_add(den1[:], den1[:], -E2)
            nc.vector.tensor_scalar_add(den2[:], den2[:], -E2)

            dd = fin.tile([128, NRT], F32, tag="dd", name="dd")
            nc.vector.tensor_mul(dd[:], den1[:], den2[:])
            lnout = fin.tile([128, NRT], F32, tag="lnout", name="lnout")
            nc.scalar.activation(lnout[:], dd[:], AF.Ln, bias=zb[:],
                                 accum_out=lnacc[:])
            iim = fin.tile([1, 1], F32, tag="iim", name="iim")
            nc.vector.tensor_scalar_mul(iim[:], ii_tot[:], -2.0 / TAU)
            nc.vector.tensor_add(lnacc[0:1, :], lnacc[0:1, :], iim[:])
            ptot = fps.tile([1, 1], F32, tag="ptot", name="ptot")
            nc.tensor.matmul(ptot[:], ones_f[:], lnacc[:], start=True,
                             stop=True)
            res = fin.tile([1, 1], F32, tag="res", name="res")
            nc.vector.tensor_copy(res[:], ptot[:])
            nc.sync.dma_start(out.ap()[:, :], res[:])

    nc.compile()
    return nc


_CACHE = {}


def _compiled(N=N_FULL, D=D_FULL, n_cores=N_CORES):
    key = (N, D, n_cores)
    if key not in _CACHE:
        _CACHE[key] = build_nc(N, D, n_cores)
    return _CACHE[key]


def make_in_maps(H_1, H_2, n_cores=N_CORES):
    H1 = np.asarray(H_1, dtype=np.float32)
    H2 = np.asarray(H_2, dtype=np.float32)
    HT1 = np.ascontiguousarray(H1.astype(ml_dtypes.bfloat16).T)
    HT2 = np.ascontiguousarray(H2.astype(ml_dtypes.bfloat16).T)

    def blk(HT, b):
        return HT[:, (b % 8) * R:(b % 8) * R + R]

    maps = []
    for c in range(n_cores):
        sel1 = np.zeros(128, np.float32)
        sel2 = np.zeros(128, np.float32)
        for c2 in range(8):
            b1 = c2 * NV  # rows: v 0-7 = s12 col b, 8-10 = s11 d, 11-13 = s22
            sel2[b1 + c] = 1.0
            for di, d in enumerate((1, 2, 3)):
                if (c2 + d) % 8 == c:
                    sel1[b1 + 8 + di] = 1.0
                    sel2[b1 + 11 + di] = 1.0
        maps.append({
            "HT2": HT2,
            "PR1": np.ascontiguousarray(
                np.concatenate([blk(HT1, c + j) for j in range(4)], axis=1)),
            "PR2": np.ascontiguousarray(
                np.concatenate([blk(HT2, c + j) for j in range(4)], axis=1)),
            "PDR1": np.ascontiguousarray(blk(HT1, c + 4)),
            "PDR2": np.ascontiguousarray(blk(HT2, c + 4)),
            "SEL1": sel1.astype(ml_dtypes.bfloat16).reshape(128, 1),
            "SEL2": sel2.astype(ml_dtypes.bfloat16).reshape(128, 1),
        })
    return maps


def kernel(H_1, H_2):
    N, D = H_1.shape
    nc = _compiled(N, D, N_CORES)
    in_maps = make_in_maps(H_1, H_2, N_CORES)
    res = run_bass_kernel_spmd(nc, in_maps, core_ids=list(range(N_CORES)))
    total = sum(float(r["out"][0, 0]) for r in res.results)
    return np.float32(total / (2.0 * N))


# revision 25
# speedup vs baseline: 1.1695x; 1.0130x over previous
"""Trainium2 8-core kernel for the paired contrastive (NT-Xent-like) loss.

Math (tau=0.5, N=8192, D=256):
    z1 = l2norm(H_1), z2 = l2norm(H_2)
    den1_i = sum_j exp(z1.z1/t) + sum_j exp(z1.z2/t) - e^2
    den2_i = sum_j exp(z2.z2/t) + sum_j exp(z2.z1/t) - e^2
    loss = (1/2N) * sum_i [ ln(den1_i) + ln(den2_i) - 2*(z1_i.z2_i)/t ]

Scheme:
  * S11/S22 are symmetric: only the upper block-triangle is computed
    (36 of 64 blocks each; with S12's 64 the global work is 136 blocks
    instead of 192 -- exp and matmul both).
  * SPMD circulant assignment: core c computes S11 blocks (c, c+d mod 8)
    d=0..3, S22 likewise, S12 row c, plus one diff-4 block: S11 (c, c+4)
    for c<4 else S22 (c, c-4) -- fed uniformly via per-core input panels.
  * Off-diagonal symmetric blocks contribute row-sums (own rows, local
    via ACT accum_out) and column-sums (other cores' rows).  Column-sum
    vectors are AllGather'ed (split in two so most hides under compute);
    each core assembles its denominators with 0/1 selection vectors
    (per-core input data) via one small matmul, sidestepping SPMD's
    compile-time addressing of "(c+d) mod 8".
  * Matmuls are fp8e4 DoubleRow (K=256 per instruction at 2x rate);
    embeddings are scaled by 16 pre-quantization, the exp activation
    scale folds 1/256 back out.
  * Normalization is pipelined in three phases per tensor -- per-chunk
    sumsq (Pool square + PE ones-matmul), one batched rsqrt across all
    chunks (partition-parallel), per-chunk broadcast+quantize -- and
    software-interleaved with the exp streams.
"""

import math

import numpy as np
import ml_dtypes

import concourse.bass as bass
import concourse.tile as tile
from concourse import bacc, mybir
from concourse.bass_utils import run_bass_kernel_spmd

F32 = mybir.dt.float32
BF16 = mybir.dt.bfloat16
FP8 = mybir.dt.float8e4
AF = mybir.ActivationFunctionType
ALU = mybir.AluOpType
AX = mybir.AxisListType
DR = mybir.MatmulPerfMode.DoubleRow

TAU = 0.5
E2 = math.exp(1.0 / TAU)
QS = 16.0                      # fp8 pre-quantization scale
ESC = 1.0 / (QS * QS * TAU)    # exp() activation scale: undo QS^2, apply 1/tau

N_FULL, D_FULL, N_CORES = 8192, 256, 8
R = N_FULL // N_CORES          # 1024 rows per block
NRT = R // 128                 # 8 row-tiles per block
CH = 512
NV = 14                        # AllGather vectors per core


def build_nc(N=N_FULL, D=D_FULL, n_cores=N_CORES):
    assert (N, D, n_cores) == (N_FULL, D_FULL, N_CORES)

    nc = bacc.Bacc("TRN2", target_bir_lowering=False, debug=False,
                   num_devices=n_cores)

    ht2 = nc.dram_tensor("HT2", [D, N], BF16, kind="ExternalInput")
    pr1 = nc.dram_tensor("PR1", [D, 4 * R], BF16, kind="ExternalInput")
    pr2 = nc.dram_tensor("PR2", [D, 4 * R], BF16, kind="ExternalInput")
    pd1 = nc.dram_tensor("PDR1", [D, R], BF16, kind="ExternalInput")
    pd2 = nc.dram_tensor("PDR2", [D, R], BF16, kind="ExternalInput")
    sel1_in = nc.dram_tensor("SEL1", [128, 1], BF16, kind="ExternalInput")
    sel2_in = nc.dram_tensor("SEL2", [128, 1], BF16, kind="ExternalInput")
    out = nc.dram_tensor("out", [1, 1], F32, kind="ExternalOutput")

    with tile.TileContext(nc) as tc, \
         tc.tile_pool(name="persist", bufs=1) as per, \
         tc.tile_pool(name="escp", bufs=5) as escp, \
         tc.tile_pool(name="agw", bufs=4) as agw, \
         tc.tile_pool(name="dram", bufs=1, space="DRAM") as dram:
        # --- persistent tensors ---------------------------------------
        z2q = per.tile([128, 2, N], FP8, tag="z2q", name="z2q")
        p1q = per.tile([128, 2, 4 * R], FP8, tag="p1q", name="p1q")
        p2q = per.tile([128, 2, 4 * R], FP8, tag="p2q", name="p2q")
        d1q = per.tile([128, 2, R], FP8, tag="d1q", name="d1q")
        d2q = per.tile([128, 2, R], FP8, tag="d2q", name="d2q")
        ca12 = per.tile([128, N], BF16, tag="ca12", name="ca12")
        ca11 = per.tile([128, 4 * R], BF16, tag="ca11", name="ca11")
        ca22 = per.tile([128, 4 * R], BF16, tag="ca22", name="ca22")
        acc1 = per.tile([128, NRT, 8], F32, tag="acc1", name="acc1")
        acc2 = per.tile([128, NRT, 4], F32, tag="acc2", name="acc2")
        sel1 = per.tile([128, 1], BF16, tag="sel1", name="sel1")
        sel2 = per.tile([128, 1], BF16, tag="sel2", name="sel2")
        ii_tot = per.tile([1, 1], F32, tag="ii_tot", name="ii_tot")
        lnacc = per.tile([128, 1], F32, tag="lnacc", name="lnacc")
        ones_k = per.tile([128, 1], BF16, tag="ones_k", name="ones_k")
        ones_b = per.tile([1, 128], BF16, tag="ones_b", name="ones_b")
        ones_f = per.tile([128, 1], F32, tag="ones_f", name="ones_f")
        zb = per.tile([128, 1], F32, tag="zb", name="zb")
        ag_in = dram.tile([NV, R], BF16, tag="ag_in", name="ag_in")
        ag_out = dram.tile([8 * NV, R], BF16, tag="ag_out", name="ag_out")
        ds1 = dram.tile([R], F32, tag="ds1", name="ds1")
        ds2 = dram.tile([R], F32, tag="ds2", name="ds2")

        nc.gpsimd.memset(ones_k[:], 1.0)
        nc.gpsimd.memset(ones_b[:], 1.0)
        nc.gpsimd.memset(ones_f[:], 1.0)
        nc.gpsimd.memset(zb[:], 0.0)

        nc.sync.dma_start(sel1[:], sel1_in.ap()[:, :])
        nc.sync.dma_start(sel2[:], sel2_in.ap()[:, :])

        spool_cm = tc.tile_pool(name="spool", bufs=2, space="PSUM")
        spool = spool_cm.__enter__()

        def ship_vec(agt, v, src, coff):
            """partition-reduce colacc [128,1024] slice -> ag row v."""
            pc = spool.tile([128, 2048], F32, tag="sg", name="sg")
            for h in range(2):
                nc.tensor.matmul(pc[0:1, h * CH:(h + 1) * CH], ones_k[:],
                                 src[:, coff + h * CH:coff + (h + 1) * CH],
                                 start=True, stop=True)
            agv = agw.tile([1, R], BF16, tag="agv", name="agv")
            nc.vector.tensor_copy(agv[:], pc[0:1, 0:R])
            nc.scalar.dma_start(agt[v:v + 1, :], agv[:])

        def stream_group(LHS, RHS, rt, c0, G, ca, acc, acol):
            """one (row-tile, col-group): matmul+exp+rowsum-accum; colacc
            into ca (None = rowsum-only block)."""
            lhs = LHS[:, :, bass.ts(rt, 128)]
            sg = spool.tile([128, G], F32, tag="sg", name="sg")
            for h in range(G // CH):
                col = slice(c0 + h * CH, c0 + (h + 1) * CH)
                nc.tensor.matmul(sg[:, h * CH:(h + 1) * CH], lhs,
                                 RHS[:, :, col], start=True, stop=True,
                                 perf_mode=DR)
            esc = escp.tile([128, G], BF16, tag="esc", name="esc")
            nc.scalar.activation(esc[:], sg[:], AF.Exp, bias=zb[:], scale=ESC,
                                 accum_out=acc[:, rt, acol:acol + 1])
            if ca is None:
                return
            if rt == 0:
                nc.vector.tensor_copy(ca[:, c0:c0 + G], esc[:])
            else:
                nc.vector.tensor_add(ca[:, c0:c0 + G],
                                     ca[:, c0:c0 + G], esc[:])

        with tc.tile_pool(name="stage", bufs=1) as stg, \
             tc.tile_pool(name="work", bufs=2) as work:
            p1b = stg.tile([128, 2, 4 * R], BF16, tag="p1b", name="p1b")
            p2b = stg.tile([128, 2, 4 * R], BF16, tag="p2b", name="p2b")
            z2b = stg.tile([128, 2, N], BF16, tag="z2b", name="z2b")
            db1 = stg.tile([128, 2, R], BF16, tag="db1", name="db1")
            db2 = stg.tile([128, 2, R], BF16, tag="db2", name="db2")
            zi1 = stg.tile([128, 2, R], BF16, tag="zi1", name="zi1")
            zi2 = stg.tile([128, 2, R], BF16, tag="zi2", name="zi2")
            rvu = stg.tile([1, 4, CH], F32, tag="rvu", name="rvu")
            ssn = {"p1": stg.tile([8, CH], F32, tag="ssn_p1", name="ssn_p1"),
                   "z2": stg.tile([16, CH], F32, tag="ssn_z2", name="ssn_z2"),
                   "p2": stg.tile([8, CH], F32, tag="ssn_p2", name="ssn_p2"),
                   "dd": stg.tile([4, CH], F32, tag="ssn_dd", name="ssn_dd")}
            rvq = {"p1": stg.tile([8, CH], BF16, tag="rvq_p1", name="rvq_p1"),
                   "z2": stg.tile([16, CH], BF16, tag="rvq_z2",
                                  name="rvq_z2"),
                   "p2": stg.tile([8, CH], BF16, tag="rvq_p2", name="rvq_p2"),
                   "dd": stg.tile([4, CH], BF16, tag="rvq_dd", name="rvq_dd")}

            # --- input loads (pr1 first, z2 split into quarters) -------
            for k in range(2):
                nc.sync.dma_start(p1b[:, k, :], pr1.ap()[bass.ts(k, 128), :])
            for k in range(2):
                for h in range(2):
                    cs = slice(h * N // 2, (h + 1) * N // 2)
                    nc.scalar.dma_start(z2b[:, k, cs],
                                        ht2.ap()[bass.ts(k, 128), cs])
            for k in range(2):
                nc.sync.dma_start(p2b[:, k, :], pr2.ap()[bass.ts(k, 128), :])
            for k in range(2):
                nc.sync.dma_start(db1[:, k, :], pd1.ap()[bass.ts(k, 128), :])
                nc.sync.dma_start(db2[:, k, :], pd2.ap()[bass.ts(k, 128), :])

            def prep_a(src, key, c, row=None, sqeng=None):
                """sumsq of chunk c -> ssn[key] row (square on sqeng, PE
                ones-matmul into a spool slot, copy+DMA out of PSUM)."""
                cs = slice(c * CH, (c + 1) * CH)
                sq = work.tile([128, 2, CH], BF16, tag="sq", name="sq")
                (sqeng or nc.gpsimd).tensor_mul(sq[:], src[:, :, cs],
                                                src[:, :, cs])
                sgp = spool.tile([128, 2048], F32, tag="sg", name="sg")
                pn = sgp[0:1, 0:CH]
                for k in range(2):
                    nc.tensor.matmul(pn, ones_k[:], sq[:, k, :],
                                     start=(k == 0), stop=(k == 1))
                r = c if row is None else row
                st = work.tile([1, CH], F32, tag="st", name="st")
                nc.vector.tensor_copy(st[:], pn)
                nc.sync.dma_start(ssn[key][r:r + 1, :], st[:])

            def prep_b(key, nrows):
                """batched rv = rsqrt(ssn)*QS in bf16 (partition-parallel)."""
                t = ssn[key]
                nc.vector.reciprocal(t[:], t[:])
                nc.scalar.activation(t[:], t[:], AF.Sqrt, bias=zb[:nrows, :])
                nc.vector.tensor_scalar_mul(rvq[key][:], t[:], QS)

            def prep_c(src, q, key, c, row=None):
                """broadcast rv row, quantize chunk c to fp8."""
                cs = slice(c * CH, (c + 1) * CH)
                r = c if row is None else row
                rst = work.tile([1, CH], BF16, tag="rst", name="rst")
                nc.scalar.dma_start(rst[:], rvq[key][r:r + 1, :])
                sgp = spool.tile([128, 2048], F32, tag="sg", name="sg")
                pb = sgp[:, CH:2 * CH]
                nc.tensor.matmul(pb, ones_b[:], rst[0:1, :],
                                 start=True, stop=True)
                for k in range(2):
                    nc.vector.tensor_mul(q[:, k, cs], src[:, k, cs], pb)

            def unit_chunk(srcb, dsti, rv_row, c):
                """zi[:,:,cs] = bf16 of src * rsqrt (unit scale, diag)."""
                cs = slice(c * CH, (c + 1) * CH)
                rst = work.tile([1, CH], BF16, tag="rst", name="rst")
                nc.vector.tensor_copy(rst[:], rvu[0:1, rv_row, :])
                sgp = spool.tile([128, 2048], F32, tag="sg", name="sg")
                pb = sgp[:, CH:2 * CH]
                nc.tensor.matmul(pb, ones_b[:], rst[0:1, :],
                                 start=True, stop=True)
                for k in range(2):
                    nc.vector.tensor_mul(dsti[:, k, cs], srcb[:, k, cs], pb)

            def ii_chunk(c):
                """ii_tot += sum over chunk of z1_i.z2_i (own rows)."""
                cs = slice(c * CH, (c + 1) * CH)
                prd = work.tile([128, 2, CH], BF16, tag="sq", name="sq")
                nc.vector.tensor_mul(prd[:], zi1[:, :, cs], zi2[:, :, cs])
                sgp = spool.tile([128, 2048], F32, tag="sg", name="sg")
                pii = sgp[0:1, 0:CH]
                for k in range(2):
                    nc.tensor.matmul(pii, ones_k[:], prd[:, k, :],
                                     start=(k == 0), stop=(k == 1))
                red = work.tile([1, 1], F32, tag="red", name="red")
                nc.vector.tensor_reduce(red[:], pii, AX.X, ALU.add)
                if c == 0:
                    nc.vector.tensor_copy(ii_tot[:], red[:])
                else:
                    nc.vector.tensor_add(ii_tot[:], ii_tot[:], red[:])

            # ---- pr1 head: full normalize of the own-row z1 panel -----
            for c in range(8):
                prep_a(p1b, "p1", c, sqeng=nc.vector)
            prep_b("p1", 8)
            nc.sync.dma_start(rvu[0:1, 0:2, :], ssn["p1"][0:2, :])
            for c in range(8):
                prep_c(p1b, p1q, "p1", c)

            # ---- s11 stream (off-diag) interleaved with z2 normalize --
            s11_groups = [(rt, g) for rt in range(NRT) for g in range(2)]
            for i, (rt, g) in enumerate(s11_groups):
                if i < 8:
                    prep_a(z2b, "z2", 2 * i, sqeng=nc.vector)
                    prep_a(z2b, "z2", 2 * i + 1, sqeng=nc.gpsimd)
                elif i == 8:
                    prep_b("z2", 16)
                if i >= 8:
                    prep_c(z2b, z2q, "z2", 2 * (i - 8))
                    prep_c(z2b, z2q, "z2", 2 * (i - 8) + 1)
                if g == 0:
                    stream_group(p1q, p1q, rt, 1024, 2048, ca11, acc1, 4)
                else:
                    stream_group(p1q, p1q, rt, 3072, 1024, ca11, acc1, 5)
            for d in (1, 2, 3):
                ship_vec(ag_in, 7 + d, ca11, d * R)

            # ---- s12 stream interleaved with pr2/pdl/pdr/diag prep ----
            s12_groups = [(rt, g) for rt in range(NRT) for g in range(4)]
            for i, (rt, g) in enumerate(s12_groups):
                if i < 4:
                    prep_a(p2b, "p2", 2 * i, sqeng=nc.vector)
                    prep_a(p2b, "p2", 2 * i + 1, sqeng=nc.gpsimd)
                elif i == 4:
                    prep_b("p2", 8)
                    nc.sync.dma_start(rvu[0:1, 2:4, :], ssn["p2"][0:2, :])
                if 4 <= i < 8:
                    prep_c(p2b, p2q, "p2", 2 * (i - 4))
                    prep_c(p2b, p2q, "p2", 2 * (i - 4) + 1)
                elif i == 8:
                    prep_a(db1, "dd", 0, row=0)
                    prep_a(db1, "dd", 1, row=1)
                elif i == 9:
                    prep_a(db2, "dd", 0, row=2)
                    prep_a(db2, "dd", 1, row=3)
                elif i == 10:
                    prep_b("dd", 4)
                elif i == 11:
                    prep_c(db1, d1q, "dd", 0, row=0)
                    prep_c(db1, d1q, "dd", 1, row=1)
                elif i == 12:
                    prep_c(db2, d2q, "dd", 0, row=2)
                    prep_c(db2, d2q, "dd", 1, row=3)
                elif i == 13:
                    unit_chunk(p1b, zi1, 0, 0)
                elif i == 14:
                    unit_chunk(p1b, zi1, 1, 1)
                elif i == 15:
                    unit_chunk(p2b, zi2, 2, 0)
                elif i == 16:
                    unit_chunk(p2b, zi2, 3, 1)
                elif i == 17:
                    ii_chunk(0)
                elif i == 18:
                    ii_chunk(1)
                stream_group(p1q, z2q, rt, g * 2048, 2048, ca12, acc1, g)
            for b in range(8):
                ship_vec(ag_in, b, ca12, b * R)

        # ---- s22 stream (off-diag) ------------------------------------
        for rt in range(NRT):
            stream_group(p2q, p2q, rt, 1024, 2048, ca22, acc2, 0)
            stream_group(p2q, p2q, rt, 3072, 1024, ca22, acc2, 1)
        for d in (1, 2, 3):
            ship_vec(ag_in, 10 + d, ca22, d * R)
        # all 14 vectors shipped: post the single AllGather now, then
        # overlap its latency with the colsum-free local blocks.
        nc.gpsimd.collective_compute(
            "AllGather", ALU.bypass, replica_groups=[list(range(n_cores))],
            ins=[ag_in.opt()], outs=[ag_out.opt()])

        # ---- local rowsum-only blocks: s11/s22 diagonals + both diff-4
        for rt in range(NRT):
            stream_group(p1q, p1q, rt, 0, 1024, None, acc1, 6)
            stream_group(p2q, p2q, rt, 0, 1024, None, acc2, 2)
            stream_group(p1q, d1q, rt, 0, 1024, None, acc1, 7)
            stream_group(p2q, d2q, rt, 0, 1024, None, acc2, 3)
        spool_cm.__exit__(None, None, None)

        # ============ selection + final ================================
        with tc.tile_pool(name="fin", bufs=1) as fin, \
             tc.tile_pool(name="fps", bufs=1, space="PSUM") as fps:
            M = fin.tile([128, R], BF16, tag="M", name="M")
            nc.gpsimd.memset(M[96:128, :], 0.0)
            nc.sync.dma_start(M[0:8 * NV, :], ag_out[:, :])
            for i, (seli, dsx) in enumerate(((sel1, ds1), (sel2, ds2))):
                pd = fps.tile([1, R], F32, tag="pd", name="pd")
                for h in range(2):
                    nc.tensor.matmul(pd[:, h * CH:(h + 1) * CH], seli[:],
                                     M[:, h * CH:(h + 1) * CH],
                                     start=True, stop=True)
                sc = fin.tile([1, R], F32, tag=f"sc{i}", name="sc")
                nc.vector.tensor_copy(sc[:], pd[:])
                nc.sync.dma_start(dsx[:], sc[:])
            dn1 = fin.tile([128, NRT], F32, tag="dn1", name="dn1")
            dn2 = fin.tile([128, NRT], F32, tag="dn2", name="dn2")
            nc.sync.dma_start(dn1[:], ds1.rearrange("(t p) -> p t", p=128))
            nc.sync.dma_start(dn2[:], ds2.rearrange("(t p) -> p t", p=128))

            den1 = fin.tile([128, NRT], F32, tag="den1", name="den1")
            den2 = fin.tile([128, NRT], F32, tag="den2", name="den2")
            for rt in range(NRT):
                nc.vector.tensor_reduce(den1[:, rt:rt + 1], acc1[:, rt, :],
                                        AX.X, ALU.add)
                nc.vector.tensor_reduce(den2[:, rt:rt + 1], acc2[:, rt, :],
                                        AX.X, ALU.add)
            nc.vector.tensor_add(den1[:], den1[:], dn1[:])
            nc.vector.tensor_add(den2[:], den2[:], dn2[:])
            nc.vector.tensor_scalar_add(den1[:], den1[:], -E2)
            nc.vector.tensor_scalar_add(den2[:], den2[:], -E2)

            dd = fin.tile([128, NRT], F32, tag="dd", name="dd")
            nc.vector.tensor_mul(dd[:], den1[:], den2[:])
            lnout = fin.tile([128, NRT], F32, tag="lnout", name="lnout")
            nc.scalar.activation(lnout[:], dd[:], AF.Ln, bias=zb[:],
                                 accum_out=lnacc[:])
            iim = fin.tile([1, 1], F32, tag="iim", name="iim")
            nc.vector.tensor_scalar_mul(iim[:], ii_tot[:], -2.0 / TAU)
            nc.vector.tensor_add(lnacc[0:1, :], lnacc[0:1, :], iim[:])
            ptot = fps.tile([1, 1], F32, tag="ptot", name="ptot")
            nc.tensor.matmul(ptot[:], ones_f[:], lnacc[:], start=True,
                             stop=True)
            res = fin.tile([1, 1], F32, tag="res", name="res")
            nc.vector.tensor_copy(res[:], ptot[:])
            nc.sync.dma_start(out.ap()[:, :], res[:])

    nc.compile()
    return nc


_CACHE = {}


def _compiled(N=N_FULL, D=D_FULL, n_cores=N_CORES):
    key = (N, D, n_cores)
    if key not in _CACHE:
        _CACHE[key] = build_nc(N, D, n_cores)
    return _CACHE[key]


def make_in_maps(H_1, H_2, n_cores=N_CORES):
    H1 = np.asarray(H_1, dtype=np.float32)
    H2 = np.asarray(H_2, dtype=np.float32)
    HT1 = np.ascontiguousarray(H1.astype(ml_dtypes.bfloat16).T)
    HT2 = np.ascontiguousarray(H2.astype(ml_dtypes.bfloat16).T)

    def blk(HT, b):
        return HT[:, (b % 8) * R:(b % 8) * R + R]

    maps = []
    for c in range(n_cores):
        sel1 = np.zeros(128, np.float32)
        sel2 = np.zeros(128, np.float32)
        for c2 in range(8):
            b1 = c2 * NV  # rows: v 0-7 = s12 col b, 8-10 = s11 d, 11-13 = s22
            sel2[b1 + c] = 1.0
            for di, d in enumerate((1, 2, 3)):
                if (c2 + d) % 8 == c:
                    sel1[b1 + 8 + di] = 1.0
                    sel2[b1 + 11 + di] = 1.0
        maps.append({
            "HT2": HT2,
            "PR1": np.ascontiguousarray(
                np.concatenate([blk(HT1, c + j) for j in range(4)], axis=1)),
            "PR2": np.ascontiguousarray(
                np.concatenate([blk(HT2, c + j) for j in range(4)], axis=1)),
            "PDR1": np.ascontiguousarray(blk(HT1, c + 4)),
            "PDR2": np.ascontiguousarray(blk(HT2, c + 4)),
            "SEL1": sel1.astype(ml_dtypes.bfloat16).reshape(128, 1),
            "SEL2": sel2.astype(ml_dtypes.bfloat16).reshape(128, 1),
        })
    return maps


def kernel(H_1, H_2):
    N, D = H_1.shape
    nc = _compiled(N, D, N_CORES)
    in_maps = make_in_maps(H_1, H_2, N_CORES)
    res = run_bass_kernel_spmd(nc, in_maps, core_ids=list(range(N_CORES)))
    total = sum(float(r["out"][0, 0]) for r in res.results)
    return np.float32(total / (2.0 * N))
